# revision 48
# baseline (speedup 1.0000x reference)
"""Bass/Trainium2 kernel for a 2-layer GCN with knowledge-enhanced output
(nn_KeGNN): y = log_softmax(relu(GCN2(relu(GCN1(x))) + P*K*U)).

Distribution strategy (8 NeuronCores, SPMD one NEFF):
  * Nodes are partitioned into 8 contiguous shards (12500 each); core c owns
    the edges whose *destination* is in shard c and produces the output rows
    of its shard.
  * GCN normalization is folded node-wise: with dinv = 1/sqrt(deg),
    table = dinv * (H @ W) gives messages, and the aggregated sum is scaled
    by dinv[dst].  The per-edge segment-sum becomes:
       agg[dst-tile] += S.T @ G        (TensorE matmul, PSUM accumulate)
    where G = dma_gather(table, src-index) and S is a 0/1 selection matrix
    built on VectorE with one is_equal against a static iota row.
  * Both layer tables are built per-shard and AllGathered (cheap, ~80us):
    layer 1 from a per-core [F, SHARD] slice of x^T, layer 2 from h1@W2 in
    the layer-1 postproc.  The same own-shard matmul pass also initializes
    agg with the self-loop contribution (dinv * own table row), so explicit
    self-loop gather tokens are not needed.
  * Source indices are int16 (hardware gather limit 32767) so the gather is
    split into 4 source blocks of 25000 nodes.  Edge tokens are grouped by
    (src-block, dst-supertile of 16 tiles) and padded to the cross-core max
    only at group granularity; each dst-tile consumes the cross-core
    min/max K-tile span of its group, with the S masks (built from per-core
    dstloc data) zeroing other cores' overhang.  One program serves all 8
    cores; per-core behavior differs only through input data.
  * Gather descriptors cost ~1.3ns each on 4 SWDGE queues (the dominant
    serial term together with DVE instruction count), so the layout
    minimizes token count: 205k tokens/core/layer vs 212.5k edges+loops.
"""

import numpy as np


# ----------------------------------------------------------------- config --
class CFG:
    N = 100000      # nodes
    F = 128         # input feature dim
    H = 64          # hidden dim
    O = 40          # output dim
    E = 1600000     # edges (without self loops)
    C = 8           # cores
    NBLK = 4        # src blocks (int16 gather index limit)
    CH_KT = 16      # K-tiles (of 128 tokens) per dma_gather call
                    # (2048 descriptors/call; needs DMA_SCRATCH >= 32KB)
    SLAB = 2048     # nodes per xT slab load in table1 build
    DMA_SCRATCH = 32768   # per-partition SWDGE desc-ring carveout bytes
    STG = 14        # dst-tiles per staged DRAM write in postproc

    def __init__(self, **kw):
        for k, v in kw.items():
            setattr(self, k, v)
        assert self.N % self.C == 0
        self.SHARD = self.N // self.C
        self.NT = -(-self.SHARD // 128)          # dst tiles per core
        self.LASTV = self.SHARD - (self.NT - 1) * 128  # valid rows in last tile
        assert self.N % self.NBLK == 0
        self.BLK = self.N // self.NBLK
        assert self.BLK <= 32767
        self.NBT = -(-self.BLK // 128)           # node tiles per block
        self.HP = 64                             # padded layer-2 table width
        assert self.O <= self.HP


def _cdiv(a, b):
    return -(-a // b)


# ----------------------------------------------------- host preprocessing --
class Layout:
    """Cross-core-common token layout.

    Tokens are grouped by (src-block b, dst-supertile T, dst-tile t); each
    (b, t) group gets the cross-core max token count (ctok), supertile
    streams are padded to multiples of 128 so K-tiles never span supertiles.
    dstloc values are relative to the supertile base (< GT*128).
    """

    GT = 16  # dst tiles per supertile

    def __init__(self, cfg: CFG, ctok):
        self.ctok = ctok  # [NBLK, NT] common per-(b,t) token counts
        NT, NBLK = cfg.NT, cfg.NBLK
        self.NSUP = _cdiv(NT, self.GT)
        self.off = np.zeros((NBLK, NT), dtype=np.int64)  # global token offset
        self.nk_sup = np.zeros((NBLK, self.NSUP), dtype=np.int64)
        self.blk_kt_base = [0] * (NBLK + 1)
        pos = 0
        for b in range(NBLK):
            for T in range(self.NSUP):
                t0, t1 = T * self.GT, min((T + 1) * self.GT, NT)
                sup_len = 0
                for t in range(t0, t1):
                    self.off[b, t] = pos + sup_len
                    sup_len += int(ctok[b, t])
                sup_pad = _cdiv(sup_len, 128) * 128
                self.nk_sup[b, T] = sup_pad // 128
                pos += sup_pad
            self.blk_kt_base[b + 1] = pos // 128
        self.nktot = pos // 128
        self.ntok = pos


class Layout3:
    """V3 token layout: edge tokens only (self-loops handled densely),
    grouped by (src-block b, dst-supertile T) with padding at group level;
    per dst-tile K-ranges are the cross-core min/max span (S masks the
    out-of-range tokens of other cores)."""

    def __init__(self, cfg: CFG, cnt, order="bT", GT=16):
        # cnt: [C, NBLK, NT] per-core per-(block, dst-tile) edge counts
        # order "bT": groups laid out block-major (gathers per block,
        #             consume per (b, t), 4 agg adds per tile).
        # order "Tb": supertile-major (per tile, one PSUM accumulation
        #             across all 4 blocks, single agg add).
        C, NBLK, NT = cfg.C, cfg.NBLK, cfg.NT
        self.GT = GT
        NSUP = _cdiv(NT, GT)
        self.NSUP = NSUP
        self.order = order
        self.off_group = np.zeros((NBLK, NSUP), dtype=np.int64)
        self.glen = np.zeros((NBLK, NSUP), dtype=np.int64)
        self.kr = {}          # (b, t) -> (k0, k1) global K-tile span or None
        self.blk_kt_base = [0] * (NBLK + 1)
        if order == "bT":
            pairs = [(b, T) for b in range(NBLK) for T in range(NSUP)]
        else:
            pairs = [(b, T) for T in range(NSUP) for b in range(NBLK)]
        pos = 0
        for b, T in pairs:
            t0, t1 = T * GT, min((T + 1) * GT, NT)
            g = cnt[:, b, t0:t1]                      # [C, tiles]
            pre = np.concatenate(
                [np.zeros((C, 1), np.int64), np.cumsum(g, axis=1)],
                axis=1)
            gmax = int(pre[:, -1].max())
            glen = _cdiv(gmax, 128) * 128
            self.off_group[b, T] = pos
            self.glen[b, T] = glen
            for ti in range(t1 - t0):
                mn = int(pre[:, ti].min())
                mx = int(pre[:, ti + 1].max())
                if mx > mn:
                    self.kr[(b, t0 + ti)] = ((pos + mn) // 128,
                                             (pos + mx - 1) // 128)
                else:
                    self.kr[(b, t0 + ti)] = None
            pos += glen
            if order == "bT" and T == NSUP - 1:
                self.blk_kt_base[b + 1] = pos // 128
        self.nktot = pos // 128
        self.ntok = pos


def _preprocess_v3(edge_index, cfg: CFG, order="bT"):
    """V3: edges only (no self-loop tokens), supertile-level padding.

    Returns (deg, Layout3, per_core)."""
    N, C, NBLK = cfg.N, cfg.C, cfg.NBLK
    NT, SHARD, BLK = cfg.NT, cfg.SHARD, cfg.BLK

    src = np.asarray(edge_index[0], dtype=np.int64)
    dst = np.asarray(edge_index[1], dtype=np.int64)
    loops_deg = np.concatenate([dst, np.arange(N, dtype=np.int64)])
    deg = np.bincount(loops_deg, minlength=N).astype(np.float32)

    core = dst // SHARD
    tloc = (dst % SHARD) // 128
    blk = src // BLK
    key = (core * NBLK + blk) * NT + tloc
    sort = np.argsort(key, kind="stable")
    s_src = src[sort]
    s_dst = dst[sort]

    ngroups = C * NBLK * NT
    cnt = np.bincount(key, minlength=ngroups).reshape(C, NBLK, NT)
    starts = np.zeros(ngroups + 1, dtype=np.int64)
    np.cumsum(cnt.reshape(-1), out=starts[1:])

    lay = Layout3(cfg, cnt, order=order, GT=(8 if order == "Tb" else 16))
    GT = lay.GT

    per_core = []
    for c in range(C):
        idx_stream = np.zeros(lay.ntok, dtype=np.int16)
        dloc_stream = np.full(lay.ntok, 9999.0, dtype=np.float32)
        for b in range(NBLK):
            for T in range(lay.NSUP):
                t0, t1 = T * GT, min((T + 1) * GT, NT)
                pos = int(lay.off_group[b, T])
                for t in range(t0, t1):
                    g = (c * NBLK + b) * NT + t
                    a, e = starts[g], starts[g + 1]
                    n = e - a
                    idx_stream[pos:pos + n] = (
                        s_src[a:e] - b * BLK).astype(np.int16)
                    dloc_stream[pos:pos + n] = (
                        s_dst[a:e] - (c * SHARD + T * GT * 128)
                    ).astype(np.float32)
                    pos += n
        idx_rep = np.ascontiguousarray(
            np.tile(idx_stream.reshape(-1, 16).T, (8, 1)))
        dloc_w = np.ascontiguousarray(
            dloc_stream.reshape(-1, 128).T).astype(np.float16)
        per_core.append({"idx": idx_rep, "dloc": dloc_w})

    return deg, lay, per_core


def _preprocess(edge_index, cfg: CFG):
    """Partition/sort edges, compute degrees, build per-core gather indices.

    Returns (deg, layout, per_core)."""
    N, C, NBLK = cfg.N, cfg.C, cfg.NBLK
    NT, SHARD, BLK = cfg.NT, cfg.SHARD, cfg.BLK

    loops = np.arange(N, dtype=np.int64)
    src = np.concatenate([np.asarray(edge_index[0], dtype=np.int64), loops])
    dst = np.concatenate([np.asarray(edge_index[1], dtype=np.int64), loops])
    deg = np.bincount(dst, minlength=N).astype(np.float32)

    core = dst // SHARD
    tloc = (dst % SHARD) // 128
    blk = src // BLK
    key = (core * NBLK + blk) * NT + tloc
    # secondary sort by src within each group: ascending gather addresses
    # give much better HBM locality for the 256B random reads
    order = np.argsort(key * BLK + (src - blk * BLK), kind="stable")
    s_src = src[order]
    s_dst = dst[order]

    ngroups = C * NBLK * NT
    cnt = np.bincount(key, minlength=ngroups).reshape(C, NBLK, NT)
    starts = np.zeros(ngroups + 1, dtype=np.int64)
    np.cumsum(cnt.reshape(-1), out=starts[1:])

    lay = Layout(cfg, cnt.max(axis=0))
    GT = lay.GT

    per_core = []
    for c in range(C):
        idx_stream = np.zeros(lay.ntok, dtype=np.int16)
        dloc_stream = np.full(lay.ntok, 9999.0, dtype=np.float32)
        for b in range(NBLK):
            for t in range(NT):
                g = (c * NBLK + b) * NT + t
                a, e = starts[g], starts[g + 1]
                n = e - a
                pos = lay.off[b, t]
                idx_stream[pos:pos + n] = (s_src[a:e] - b * BLK).astype(np.int16)
                dloc_stream[pos:pos + n] = (
                    s_dst[a:e] - (c * SHARD + (t // GT) * GT * 128)
                ).astype(np.float32)
        idx_rep = np.ascontiguousarray(
            np.tile(idx_stream.reshape(-1, 16).T, (8, 1))
        )  # [128, ntok//16]
        dloc_w = np.ascontiguousarray(
            dloc_stream.reshape(-1, 128).T
        ).astype(np.float16)  # [128, nktot]
        per_core.append({"idx": idx_rep, "dloc": dloc_w})

    return deg, lay, per_core


def _wrap_deg(deg, cfg: CFG):
    """degB [128, NBLK*NBT] (block-wrapped, pad 1.0) and per-core degS
    [128, NT] (shard-wrapped, pad 1.0)."""
    N, NBLK, BLK, NBT = cfg.N, cfg.NBLK, cfg.BLK, cfg.NBT
    C, SHARD, NT = cfg.C, cfg.SHARD, cfg.NT
    degB = np.ones((128, NBLK * NBT), dtype=np.float32)
    for b in range(NBLK):
        for j in range(NBT):
            base = b * BLK + j * 128
            m = min(128, (b + 1) * BLK - base, N - base)
            if m > 0:
                degB[:m, b * NBT + j] = deg[base:base + m]
    degS = np.ones((C, 128, NT), dtype=np.float32)
    for c in range(C):
        for t in range(NT):
            base = c * SHARD + t * 128
            m = min(128, (c + 1) * SHARD - base)
            degS[c, :m, t] = deg[base:base + m]
    return degB, degS


# ------------------------------------------------------------ bass program --
def _build(cfg: CFG, lay: Layout, ablate=()):
    import concourse.bacc as bacc
    import concourse.mybir as mybir
    from concourse import tile

    f32 = mybir.dt.float32
    f16 = mybir.dt.float16
    i16 = mybir.dt.int16
    i32 = mybir.dt.int32
    ALU = mybir.AluOpType
    ACTF = mybir.ActivationFunctionType

    N, F, H, O, C = cfg.N, cfg.F, cfg.H, cfg.O, cfg.C
    NBLK, BLK, NBT = cfg.NBLK, cfg.BLK, cfg.NBT
    NT, SHARD, LASTV, HP = cfg.NT, cfg.SHARD, cfg.LASTV, cfg.HP
    CH_KT, SLAB, STG = cfg.CH_KT, cfg.SLAB, cfg.STG

    nktot = lay.nktot
    ntok = lay.ntok
    blk_kt_base = lay.blk_kt_base
    GT = lay.GT
    v3 = isinstance(lay, Layout3)

    nc = bacc.Bacc("TRN2", target_bir_lowering=False, debug=False,
                   num_devices=cfg.C,
                   dynamic_dma_scratch_size=cfg.DMA_SCRATCH,
                   num_swdge_queues=4)

    # ---- DRAM I/O
    shard_build_pre = v3 and "oldbuild" not in ablate
    xT_d = (None if shard_build_pre else
            nc.dram_tensor("xT", [F, N], f32, kind="ExternalInput"))
    xTs_d = (nc.dram_tensor("xTs", [F, SHARD], f32, kind="ExternalInput")
             if v3 else None)
    degB_d = (None if shard_build_pre else
              nc.dram_tensor("degB", [128, NBLK * NBT], f32,
                             kind="ExternalInput"))
    degS_d = nc.dram_tensor("degS", [128, NT], f32, kind="ExternalInput")
    idx_d = nc.dram_tensor("idx", [128, ntok // 16], i16, kind="ExternalInput")
    dloc_d = nc.dram_tensor("dloc", [128, nktot], f16, kind="ExternalInput")
    W1_d = nc.dram_tensor("W1", [F, H], f32, kind="ExternalInput")
    W2_d = nc.dram_tensor("W2", [H, O], f32, kind="ExternalInput")
    b1_d = nc.dram_tensor("b1", [1, H], f32, kind="ExternalInput")
    b2_d = nc.dram_tensor("b2", [1, O], f32, kind="ExternalInput")
    P_d = nc.dram_tensor("P", [1, O], f32, kind="ExternalInput")
    K_d = nc.dram_tensor("K", [1, O], f32, kind="ExternalInput")
    U_d = nc.dram_tensor("U", [1, O], f32, kind="ExternalInput")
    out_d = nc.dram_tensor("out", [SHARD, O], f32, kind="ExternalOutput")

    TW = 128  # f16 table row width (256B gather granule; cols >= H unused)
    TWv = 2 * TW if "elem512" in ablate else TW
    shard_build = v3 and "oldbuild" not in ablate
    if shard_build:
        t1loc = nc.dram_tensor("t1loc", [SHARD, TWv], f16)
        tab1f = nc.dram_tensor("tab1f", [N, TWv], f16, addr_space="Shared")
        tab1 = [tab1f[b * BLK: b * BLK + min(BLK, N - b * BLK), :]
                for b in range(NBLK)]
    else:
        tab1 = [
            nc.dram_tensor(f"tab1_{b}", [min(BLK, N - b * BLK), TWv], f16)
            for b in range(NBLK)
        ]
    t2loc = nc.dram_tensor("t2loc", [SHARD, TWv], f16)
    tab2 = nc.dram_tensor("tab2", [N, TWv], f16, addr_space="Shared")

    with tile.TileContext(nc, num_cores=C) as tc:
        with (
            tc.tile_pool(name="const", bufs=1) as const,
            tc.tile_pool(name="xslab", bufs=2) as xpool,
            tc.tile_pool(name="t1st", bufs=2) as t1pool,
            tc.tile_pool(name="g", bufs=16) as gpool,
            tc.tile_pool(name="s", bufs=6) as spool,
            tc.tile_pool(name="work", bufs=2) as work,
            tc.tile_pool(name="post", bufs=2) as post,
            tc.tile_pool(name="ost", bufs=2) as opool,
            tc.tile_pool(name="ps_seg", bufs=3, space="PSUM") as ps_seg,
            tc.tile_pool(name="ps_bld", bufs=1, space="PSUM") as ps_bld,
            tc.tile_pool(name="ps_tr", bufs=2, space="PSUM") as ps_tr,
            tc.tile_pool(name="ps_t2", bufs=2, space="PSUM") as ps_t2,
        ):
            # ---------------- constants / small inputs
            iota_i = const.tile([128, GT * 128], i32)
            nc.gpsimd.iota(iota_i[:, :], pattern=[[128, GT], [1, 128]],
                           base=0, channel_multiplier=0)
            IOTA16 = const.tile([128, GT * 128], f16)
            nc.vector.tensor_copy(IOTA16[:, :], iota_i[:, :])
            IOTA = IOTA16  # first 128 columns are a plain 0..127 iota row
            IDiota = const.tile([128, 128], f32)
            pidx_i = const.tile([128, 1], i32)
            nc.gpsimd.iota(pidx_i[:, :], pattern=[[0, 1]], base=0,
                           channel_multiplier=1)
            PIDX = const.tile([128, 1], f32)
            nc.vector.tensor_copy(PIDX[:, :], pidx_i[:, :])
            ID = const.tile([128, 128], f32)
            nc.vector.tensor_copy(IDiota[:, :], iota_i[:, :128])
            nc.vector.tensor_scalar(out=ID[:, :], in0=IDiota[:, :],
                                    scalar1=PIDX[:, :], scalar2=None,
                                    op0=ALU.is_equal)

            W1s = const.tile([F, H], f32)
            nc.sync.dma_start(W1s[:, :], W1_d[:, :])
            W2s = const.tile([H, O], f32)
            nc.sync.dma_start(W2s[:, :], W2_d[:, :])

            b1row = const.tile([1, H], f32)
            nc.sync.dma_start(b1row[:, :], b1_d[:, :])
            BIAS1 = const.tile([128, H], f32)
            nc.gpsimd.partition_broadcast(BIAS1[:, :], b1row[:, :])

            b2row = const.tile([1, O], f32)
            nc.sync.dma_start(b2row[:, :], b2_d[:, :])
            prow = const.tile([1, O], f32)
            nc.sync.dma_start(prow[:, :], P_d[:, :])
            krow = const.tile([1, O], f32)
            nc.sync.dma_start(krow[:, :], K_d[:, :])
            urow = const.tile([1, O], f32)
            nc.sync.dma_start(urow[:, :], U_d[:, :])
            pku = const.tile([1, O], f32)
            nc.vector.tensor_mul(pku[:, :], prow[:, :], krow[:, :])
            nc.vector.tensor_mul(pku[:, :], pku[:, :], urow[:, :])
            nc.vector.tensor_add(pku[:, :], pku[:, :], b2row[:, :])
            BIAS2 = const.tile([128, O], f32)
            nc.gpsimd.partition_broadcast(BIAS2[:, :], pku[:, :])

            if degB_d is not None:
                degB = const.tile([128, NBLK * NBT], f32)
                nc.sync.dma_start(degB[:, :], degB_d[:, :])
                dinvB = const.tile([128, NBLK * NBT], f32)
                nc.vector.reciprocal(dinvB[:, :], degB[:, :])
                nc.scalar.sqrt(dinvB[:, :], dinvB[:, :])

            degS = const.tile([128, NT], f32)
            nc.sync.dma_start(degS[:, :], degS_d[:, :])
            dinvS = const.tile([128, NT], f32)
            nc.vector.reciprocal(dinvS[:, :], degS[:, :])
            nc.scalar.sqrt(dinvS[:, :], dinvS[:, :])

            idxS = const.tile([128, ntok // 16], i16)
            nc.sync.dma_start(idxS[:, :], idx_d[:, :])
            dloc = const.tile([128, nktot], f16)
            nc.sync.dma_start(dloc[:, :], dloc_d[:, :])

            agg = const.tile([128, NT, H], f32)
            if v3:
                # Own-shard pass over dinv*(x@W1): initializes agg with the
                # self-loop table rows AND (shard_build) stages this core's
                # slice of the layer-1 table for the AllGather.
                for s0 in range(0, SHARD, SLAB):
                    w = min(SLAB, SHARD - s0)
                    xs = xpool.tile([F, SLAB], f32, tag="xss")
                    nc.sync.dma_start(xs[:, :w], xTs_d[:, s0:s0 + w])
                    st = None
                    if shard_build:
                        st = t1pool.tile([128, _cdiv(SLAB, 128), H], f16,
                                         tag="t1st")
                    nfull = 0
                    for j0 in range(0, w, 128):
                        m = min(128, w - j0)
                        t = (s0 + j0) // 128
                        ps = ps_bld.tile([128, H], f32, tag="psb")
                        if m < 128:
                            # pad rows: zero so downstream stays finite
                            nc.vector.memset(agg[:, t, :], 0.0)
                        nc.tensor.matmul(ps[:m, :], lhsT=xs[:, j0:j0 + m],
                                         rhs=W1s[:, :], start=True, stop=True)
                        nc.scalar.activation(
                            agg[:m, t, :], ps[:m, :], ACTF.Copy,
                            scale=dinvS[:m, t:t + 1])
                        if shard_build:
                            nc.scalar.activation(
                                st[:m, j0 // 128, :], ps[:m, :], ACTF.Copy,
                                scale=dinvS[:m, t:t + 1])
                            if m == 128:
                                nfull += 1
                    if shard_build:
                        if nfull:
                            dst_ap = t1loc[s0:s0 + nfull * 128, :H].rearrange(
                                "(j p) f -> p j f", p=128)
                            nc.sync.dma_start(dst_ap, st[:, :nfull, :])
                        if nfull * 128 < w:
                            mm = w - nfull * 128
                            nc.sync.dma_start(
                                t1loc[s0 + nfull * 128: s0 + w, :H],
                                st[:mm, nfull, :])
                if shard_build:
                    nc.gpsimd.collective_compute(
                        "AllGather", mybir.AluOpType.bypass,
                        replica_groups=[list(range(C))],
                        ins=[t1loc[:, :].opt()],
                        outs=[tab1f[:, :].opt()])
            else:
                nc.vector.memset(agg[:, :, :], 0.0)

            # ---------------- layer-1 message table: tab1_b = dinv*(x@W1)
            def build_table1(b):
                nodes_b = min(BLK, N - b * BLK)
                for s0 in range(0, nodes_b, SLAB):
                    w = min(SLAB, nodes_b - s0)
                    xs = xpool.tile([F, SLAB], f32, tag="xs")
                    nc.sync.dma_start(xs[:, :w],
                                      xT_d[:, b * BLK + s0: b * BLK + s0 + w])
                    st = t1pool.tile([128, _cdiv(SLAB, 128), H], f16, tag="t1st")
                    nfull = 0
                    for j0 in range(0, w, 128):
                        m = min(128, w - j0)
                        jt = (s0 + j0) // 128  # node-tile idx within block
                        ps = ps_bld.tile([128, H], f32, tag="psb")
                        nc.tensor.matmul(ps[:m, :], lhsT=xs[:, j0:j0 + m],
                                         rhs=W1s[:, :], start=True, stop=True)
                        nc.scalar.activation(
                            st[:m, j0 // 128, :], ps[:m, :], ACTF.Copy,
                            scale=dinvB[:m, b * NBT + jt: b * NBT + jt + 1])
                        if m == 128:
                            nfull += 1
                    # store staged tiles to DRAM
                    if nfull:
                        dst_ap = tab1[b][s0:s0 + nfull * 128, :H].rearrange(
                            "(j p) f -> p j f", p=128)
                        nc.sync.dma_start(dst_ap, st[:, :nfull, :])
                    if nfull * 128 < w:  # ragged tail tile of the block
                        m = w - nfull * 128
                        nc.sync.dma_start(
                            tab1[b][s0 + nfull * 128: s0 + w, :H],
                            st[:m, nfull, :])

            if "nobuild" not in ablate and not shard_build:
                for b in range(NBLK):
                    build_table1(b)

            # ---------------- gather + segment-sum matmul for one layer
            MAXKB = 8  # S-matrices built per DVE instruction
            qrot = [0]  # SWDGE queue rotation across gather calls

            def seg_layer(table_aps, uw):
                """table_aps[b]: block b's [rows, TW] f16 message rows; only
                the first uw columns are meaningful."""
                for b in range(NBLK):
                    kt_in_blk = blk_kt_base[b + 1] - blk_kt_base[b]
                    if kt_in_blk == 0:
                        continue
                    # gather chunks
                    gtiles = []
                    SP = "singlepacket" in ablate
                    for ci in range(_cdiv(kt_in_blk, CH_KT)):
                        kts = min(CH_KT, kt_in_blk - ci * CH_KT)
                        g = gpool.tile([128, CH_KT, TWv], f16, tag="g")
                        tok0 = (blk_kt_base[b] + ci * CH_KT) * 128
                        if "smallgather" in ablate:
                            nc.gpsimd.dma_gather(
                                g[:, :1, :], table_aps[b],
                                idxS[:, tok0 // 16: (tok0 + 128) // 16],
                                num_idxs=128, num_idxs_reg=128,
                                elem_size=TWv, single_packet=SP,
                                queue_num=qrot[0] % 4)
                        elif "smallreg" in ablate:
                            nc.gpsimd.dma_gather(
                                g[:, :kts, :], table_aps[b],
                                idxS[:, tok0 // 16: (tok0 + kts * 128) // 16],
                                num_idxs=kts * 128, num_idxs_reg=128,
                                elem_size=TWv, single_packet=SP,
                                queue_num=qrot[0] % 4)
                        else:
                            nc.gpsimd.dma_gather(
                                g[:, :kts, :], table_aps[b],
                                idxS[:, tok0 // 16: (tok0 + kts * 128) // 16],
                                num_idxs=kts * 128, num_idxs_reg=kts * 128,
                                elem_size=TWv, single_packet=SP,
                                queue_num=qrot[0] % 4)
                        qrot[0] += 1
                        gtiles.append(g)

                    def gslice(kglob):
                        ci, sl = divmod(kglob - blk_kt_base[b], CH_KT)
                        return gtiles[ci][:, sl, :uw]

                    # consume: per dst-tile, its token range [o0, o1) in the
                    # common layout; K-tiles at supertile boundaries are
                    # shared between adjacent dst-tiles (S masks the others).
                    if "noconsume" in ablate:
                        continue
                    for t in range(NT):
                        if v3:
                            r = lay.kr[(b, t)]
                            if r is None:
                                continue
                            k0, k1 = r
                        else:
                            ct = int(lay.ctok[b, t])
                            if ct == 0:
                                continue
                            o0 = int(lay.off[b, t])
                            o1 = o0 + ct
                            k0, k1 = o0 // 128, (o1 - 1) // 128
                        it = t % GT  # iota variant within supertile
                        ps = ps_seg.tile([128, uw], f32, tag="pss")
                        k = k0
                        while k <= k1:
                            kb = min(MAXKB, k1 + 1 - k)
                            Sb = spool.tile([128, MAXKB, 128], f16, tag="s")
                            nc.vector.tensor_tensor(
                                out=Sb[:, :kb, :],
                                in0=IOTA16[:, it * 128:(it + 1) * 128]
                                    .unsqueeze(1)
                                    .broadcast_to([128, kb, 128]),
                                in1=dloc[:, k:k + kb].unsqueeze(2)
                                    .broadcast_to([128, kb, 128]),
                                op=ALU.is_equal)
                            for j in range(kb):
                                nc.tensor.matmul(
                                    ps[:, :], lhsT=Sb[:, j, :],
                                    rhs=gslice(k + j),
                                    start=(k + j == k0),
                                    stop=(k + j == k1))
                            k += kb
                        nc.vector.tensor_add(agg[:, t, :uw],
                                             agg[:, t, :uw], ps[:, :])

            def seg_layer_tb(table_aps, uw):
                """order='Tb': per supertile, gather all 4 block groups,
                then one PSUM accumulation per dst-tile across blocks."""
                SP = "singlepacket" in ablate
                for T in range(lay.NSUP):
                    gmap = {}
                    for b in range(NBLK):
                        off = int(lay.off_group[b, T])
                        kt_grp = int(lay.glen[b, T]) // 128
                        k_base = off // 128
                        ci = 0
                        while ci < kt_grp:
                            kts = min(CH_KT, kt_grp - ci)
                            g = gpool.tile([128, CH_KT, TWv], f16, tag="g")
                            tok0 = (k_base + ci) * 128
                            nc.gpsimd.dma_gather(
                                g[:, :kts, :], table_aps[b],
                                idxS[:, tok0 // 16:
                                     (tok0 + kts * 128) // 16],
                                num_idxs=kts * 128, num_idxs_reg=kts * 128,
                                elem_size=TWv, single_packet=SP,
                                queue_num=qrot[0] % 4)
                            qrot[0] += 1
                            for s in range(kts):
                                gmap[k_base + ci + s] = (g, s)
                            ci += kts
                    if "noconsume" in ablate:
                        continue
                    t0, t1 = T * GT, min((T + 1) * GT, NT)
                    for t in range(t0, t1):
                        spans = [lay.kr[(b, t)] for b in range(NBLK)
                                 if lay.kr[(b, t)] is not None]
                        if not spans:
                            continue
                        it = t % GT
                        firstk = spans[0][0]
                        lastk = spans[-1][1]
                        ps = ps_seg.tile([128, uw], f32, tag="pss")
                        for (k0, k1) in spans:
                            k = k0
                            while k <= k1:
                                kb = min(MAXKB, k1 + 1 - k)
                                Sb = spool.tile([128, MAXKB, 128], f16,
                                                tag="s")
                                nc.vector.tensor_tensor(
                                    out=Sb[:, :kb, :],
                                    in0=IOTA16[:, it * 128:(it + 1) * 128]
                                        .unsqueeze(1)
                                        .broadcast_to([128, kb, 128]),
                                    in1=dloc[:, k:k + kb].unsqueeze(2)
                                        .broadcast_to([128, kb, 128]),
                                    op=ALU.is_equal)
                                for j in range(kb):
                                    gt_, sl = gmap[k + j]
                                    nc.tensor.matmul(
                                        ps[:, :], lhsT=Sb[:, j, :],
                                        rhs=gt_[:, sl, :uw],
                                        start=(k + j == firstk),
                                        stop=(k + j == lastk))
                                k += kb
                        nc.vector.tensor_add(agg[:, t, :uw],
                                             agg[:, t, :uw], ps[:, :])

            seg = seg_layer_tb if (v3 and lay.order == "Tb") else seg_layer

            # ---------------- layer 1
            tab1_aps = (tab1 if shard_build
                        else [tab1[b][:, :] for b in range(NBLK)])
            seg(tab1_aps, H)

            # post: h1 = relu(dinv*agg + b1); t2 = dinv*(h1@W2) padded
            def staged_store(dram, stile, grp, nt_in_grp, width):
                """store staging tile rows [grp*STG .. ) handling ragged tail"""
                t0 = grp * STG
                nfull = 0
                for tt in range(nt_in_grp):
                    if (t0 + tt) * 128 + 128 <= SHARD:
                        nfull += 1
                if nfull:
                    dst = dram[t0 * 128: t0 * 128 + nfull * 128,
                               :width].rearrange("(j p) f -> p j f", p=128)
                    nc.sync.dma_start(dst, stile[:, :nfull, :width])
                if nfull < nt_in_grp:
                    nc.sync.dma_start(
                        dram[(t0 + nfull) * 128: SHARD, :width],
                        stile[:LASTV, nfull, :width])

            for grp in range(_cdiv(NT, STG)):
                nt_in_grp = min(STG, NT - grp * STG)
                st = post.tile([128, STG, H], f16, tag="t2st")
                if H > O:
                    nc.vector.memset(st[:, :, O:], 0.0)
                for tt in range(nt_in_grp):
                    t = grp * STG + tt
                    h1 = work.tile([128, H], f32, tag="h1")
                    nc.vector.scalar_tensor_tensor(
                        out=h1[:, :], in0=agg[:, t, :],
                        scalar=dinvS[:, t:t + 1], in1=BIAS1[:, :],
                        op0=ALU.mult, op1=ALU.add)
                    nc.scalar.activation(h1[:, :], h1[:, :], ACTF.Relu)
                    pst = ps_tr.tile([H, 128], f32, tag="pstr")
                    nc.tensor.transpose(pst[:, :], h1[:, :], ID[:, :])
                    h1t = work.tile([H, 128], f32, tag="h1t")
                    nc.scalar.copy(h1t[:, :], pst[:, :])
                    ps2 = ps_t2.tile([128, O], f32, tag="pst2")
                    nc.tensor.matmul(ps2[:, :], lhsT=h1t[:, :], rhs=W2s[:, :],
                                     start=True, stop=True)
                    nc.scalar.activation(st[:, tt, :O], ps2[:, :], ACTF.Copy,
                                         scale=dinvS[:, t:t + 1])
                    if v3:
                        # L2 self-loop init: agg[:, t, :O] = dinv*(h1@W2)
                        # (own t2 table row; postproc applies dst-side dinv)
                        nc.scalar.activation(
                            agg[:, t, :O], ps2[:, :], ACTF.Copy,
                            scale=dinvS[:, t:t + 1])
                staged_store(t2loc, st, grp, nt_in_grp, H)

            # ---------------- exchange layer-2 table
            nc.gpsimd.collective_compute(
                "AllGather", mybir.AluOpType.bypass,
                replica_groups=[list(range(C))],
                ins=[t2loc[:, :].opt()],
                outs=[tab2[:, :].opt()])
            if "agx2" in ablate:  # probe: cost of one extra AllGather
                nc.gpsimd.collective_compute(
                    "AllGather", mybir.AluOpType.bypass,
                    replica_groups=[list(range(C))],
                    ins=[t2loc[:, :].opt()],
                    outs=[tab2[:, :].opt()])

            # ---------------- layer 2
            if not v3:
                nc.vector.memset(agg[:, :, :], 0.0)
            tab2_aps = [tab2[b * BLK: b * BLK + min(BLK, N - b * BLK), :]
                        for b in range(NBLK)]
            seg(tab2_aps, O)

            # post: y = relu(dinv*agg + b2 + pku); out = log_softmax(y)
            for grp in range(0 if "nopost2" not in ablate
                             else _cdiv(NT, STG), _cdiv(NT, STG)):
                nt_in_grp = min(STG, NT - grp * STG)
                st = opool.tile([128, STG, O], f32, tag="ost")
                for tt in range(nt_in_grp):
                    t = grp * STG + tt
                    y = work.tile([128, O], f32, tag="y")
                    nc.vector.scalar_tensor_tensor(
                        out=y[:, :], in0=agg[:, t, :O],
                        scalar=dinvS[:, t:t + 1], in1=BIAS2[:, :],
                        op0=ALU.mult, op1=ALU.add)
                    nc.scalar.activation(y[:, :], y[:, :], ACTF.Relu)
                    nmax = work.tile([128, 1], f32, tag="nmax")
                    nc.vector.tensor_reduce(nmax[:, :], y[:, :],
                                            axis=mybir.AxisListType.X,
                                            op=ALU.max, negate=True)
                    ex = work.tile([128, O], f32, tag="ex")
                    esum = work.tile([128, 1], f32, tag="esum")
                    nc.scalar.activation(ex[:, :], y[:, :], ACTF.Exp,
                                         bias=nmax[:, :], scale=1.0,
                                         accum_out=esum[:, :])
                    lsum = work.tile([128, 1], f32, tag="lsum")
                    nc.scalar.activation(lsum[:, :], esum[:, :], ACTF.Ln)
                    nc.vector.tensor_scalar(
                        out=st[:, tt, :], in0=y[:, :], scalar1=nmax[:, :],
                        scalar2=lsum[:, :], op0=ALU.add, op1=ALU.subtract)
                staged_store(out_d, st, grp, nt_in_grp, O)

    nc.compile()
    return nc


# ------------------------------------------------------------------ entry --
def make_in_maps(inputs, cfg, per_core, degB, degS, xT, v3):
    in_maps = []
    for c in range(cfg.C):
        m = {
            "xT": xT,
            "degB": degB,
            "degS": np.ascontiguousarray(degS[c]),
            "idx": per_core[c]["idx"],
            "dloc": per_core[c]["dloc"],
            "W1": np.asarray(inputs["W1"], np.float32),
            "W2": np.asarray(inputs["W2"], np.float32),
            "b1": np.asarray(inputs["b1"], np.float32).reshape(1, -1),
            "b2": np.asarray(inputs["b2"], np.float32).reshape(1, -1),
            "P": np.asarray(inputs["P"], np.float32).reshape(1, -1),
            "K": np.asarray(inputs["K"], np.float32).reshape(1, -1),
            "U": np.asarray(inputs["U"], np.float32).reshape(1, -1),
        }
        if v3:
            m["xTs"] = np.ascontiguousarray(
                xT[:, c * cfg.SHARD:(c + 1) * cfg.SHARD])
        in_maps.append(m)
    return in_maps


def prepare_and_run(inputs, cfg=None, trace=False, v3=True, **run_kwargs):
    """Preprocess, build, run on 8 cores.  Returns (out, BassKernelResults)."""
    from concourse.bass_utils import run_bass_kernel_spmd

    cfg = cfg or CFG()
    x = np.asarray(inputs["x"], dtype=np.float32)
    edge_index = np.asarray(inputs["edge_index"])

    pre = _preprocess_v3 if v3 else _preprocess
    deg, lay, per_core = pre(edge_index, cfg)
    degB, degS = _wrap_deg(deg, cfg)
    xT = np.ascontiguousarray(x.T)

    nc = _build(cfg, lay)

    in_maps = make_in_maps(inputs, cfg, per_core, degB, degS, xT, v3)
    res = run_bass_kernel_spmd(nc, in_maps, core_ids=list(range(cfg.C)),
                               trace=trace, **run_kwargs)
    out = np.concatenate([res.results[c]["out"] for c in range(cfg.C)], axis=0)
    return out.astype(np.float32), res


def kernel(**inputs):
    out, _ = prepare_and_run(inputs)
    return out


if __name__ == "__main__":
    import reference

    inputs = {k: np.asarray(v) for k, v in reference.setup_inputs().items()}
    got = kernel(**inputs)
    want = np.asarray(reference.reference(**inputs))
    err = np.abs(got - want).max() / max(np.abs(want).max(), 1e-9)
    print("rel err:", err)



# revision 49
# speedup vs baseline: 1.1738x; 1.1738x over previous
"""Bass/Trainium2 kernel for a 2-layer GCN with knowledge-enhanced output
(nn_KeGNN): y = log_softmax(relu(GCN2(relu(GCN1(x))) + P*K*U)).

Distribution strategy (8 NeuronCores, SPMD one NEFF):
  * Nodes are partitioned into 8 contiguous shards (12500 each); core c owns
    the edges whose *destination* is in shard c and produces the output rows
    of its shard.
  * GCN normalization is folded node-wise: with dinv = 1/sqrt(deg),
    table = dinv * (H @ W) gives messages, and the aggregated sum is scaled
    by dinv[dst].  The per-edge segment-sum becomes:
       agg[dst-tile] += S.T @ G        (TensorE matmul, PSUM accumulate)
    where G = dma_gather(table, src-index) and S is a 0/1 selection matrix
    built on VectorE with one is_equal against a static iota row.
  * Both layer tables are built per-shard and AllGathered (cheap, ~80us):
    layer 1 from a per-core [F, SHARD] slice of x^T, layer 2 from h1@W2 in
    the layer-1 postproc.  The same own-shard matmul pass also initializes
    agg with the self-loop contribution (dinv * own table row), so explicit
    self-loop gather tokens are not needed.
  * Source indices are int16 (hardware gather limit 32767) so the gather is
    split into 4 source blocks of 25000 nodes.  Edge tokens are grouped by
    (src-block, dst-supertile of 16 tiles) and padded to the cross-core max
    only at group granularity; each dst-tile consumes the cross-core
    min/max K-tile span of its group, with the S masks (built from per-core
    dstloc data) zeroing other cores' overhang.  One program serves all 8
    cores; per-core behavior differs only through input data.
  * Gather descriptors cost ~1.3ns each on 4 SWDGE queues (the dominant
    serial term together with DVE instruction count), so the layout
    minimizes token count: 205k tokens/core/layer vs 212.5k edges+loops.
"""

import numpy as np


# ----------------------------------------------------------------- config --
class CFG:
    N = 100000      # nodes
    F = 128         # input feature dim
    H = 64          # hidden dim
    O = 40          # output dim
    E = 1600000     # edges (without self loops)
    C = 8           # cores
    NBLK = 4        # src blocks (int16 gather index limit)
    CH_KT = 16      # K-tiles (of 128 tokens) per dma_gather call
                    # (2048 descriptors/call; needs DMA_SCRATCH >= 32KB)
    SLAB = 2048     # nodes per xT slab load in table1 build
    DMA_SCRATCH = 32768   # per-partition SWDGE desc-ring carveout bytes
    STG = 25        # dst-tiles per staged DRAM write in postproc

    def __init__(self, **kw):
        for k, v in kw.items():
            setattr(self, k, v)
        assert self.N % self.C == 0
        self.SHARD = self.N // self.C
        self.NT = -(-self.SHARD // 128)          # dst tiles per core
        self.LASTV = self.SHARD - (self.NT - 1) * 128  # valid rows in last tile
        assert self.N % self.NBLK == 0
        self.BLK = self.N // self.NBLK
        assert self.BLK <= 32767
        self.NBT = -(-self.BLK // 128)           # node tiles per block
        self.HP = 64                             # padded layer-2 table width
        assert self.O <= self.HP


def _cdiv(a, b):
    return -(-a // b)


# ----------------------------------------------------- host preprocessing --
class Layout:
    """Cross-core-common token layout.

    Tokens are grouped by (src-block b, dst-supertile T, dst-tile t); each
    (b, t) group gets the cross-core max token count (ctok), supertile
    streams are padded to multiples of 128 so K-tiles never span supertiles.
    dstloc values are relative to the supertile base (< GT*128).
    """

    GT = 16  # dst tiles per supertile

    def __init__(self, cfg: CFG, ctok):
        self.ctok = ctok  # [NBLK, NT] common per-(b,t) token counts
        NT, NBLK = cfg.NT, cfg.NBLK
        self.NSUP = _cdiv(NT, self.GT)
        self.off = np.zeros((NBLK, NT), dtype=np.int64)  # global token offset
        self.nk_sup = np.zeros((NBLK, self.NSUP), dtype=np.int64)
        self.blk_kt_base = [0] * (NBLK + 1)
        pos = 0
        for b in range(NBLK):
            for T in range(self.NSUP):
                t0, t1 = T * self.GT, min((T + 1) * self.GT, NT)
                sup_len = 0
                for t in range(t0, t1):
                    self.off[b, t] = pos + sup_len
                    sup_len += int(ctok[b, t])
                sup_pad = _cdiv(sup_len, 128) * 128
                self.nk_sup[b, T] = sup_pad // 128
                pos += sup_pad
            self.blk_kt_base[b + 1] = pos // 128
        self.nktot = pos // 128
        self.ntok = pos


class Layout3:
    """V3 token layout: edge tokens only (self-loops handled densely),
    grouped by (src-block b, dst-supertile T) with padding at group level;
    per dst-tile K-ranges are the cross-core min/max span (S masks the
    out-of-range tokens of other cores)."""

    def __init__(self, cfg: CFG, cnt, order="bT", GT=16):
        # cnt: [C, NBLK, NT] per-core per-(block, dst-tile) edge counts
        # order "bT": groups laid out block-major (gathers per block,
        #             consume per (b, t), 4 agg adds per tile).
        # order "Tb": supertile-major (per tile, one PSUM accumulation
        #             across all 4 blocks, single agg add).
        C, NBLK, NT = cfg.C, cfg.NBLK, cfg.NT
        self.GT = GT
        NSUP = _cdiv(NT, GT)
        self.NSUP = NSUP
        self.order = order
        self.off_group = np.zeros((NBLK, NSUP), dtype=np.int64)
        self.glen = np.zeros((NBLK, NSUP), dtype=np.int64)
        self.kr = {}          # (b, t) -> (k0, k1) global K-tile span or None
        self.blk_kt_base = [0] * (NBLK + 1)
        if order == "bT":
            pairs = [(b, T) for b in range(NBLK) for T in range(NSUP)]
        else:
            pairs = [(b, T) for T in range(NSUP) for b in range(NBLK)]
        pos = 0
        for b, T in pairs:
            t0, t1 = T * GT, min((T + 1) * GT, NT)
            g = cnt[:, b, t0:t1]                      # [C, tiles]
            pre = np.concatenate(
                [np.zeros((C, 1), np.int64), np.cumsum(g, axis=1)],
                axis=1)
            gmax = int(pre[:, -1].max())
            glen = _cdiv(gmax, 128) * 128
            self.off_group[b, T] = pos
            self.glen[b, T] = glen
            for ti in range(t1 - t0):
                mn = int(pre[:, ti].min())
                mx = int(pre[:, ti + 1].max())
                if mx > mn:
                    self.kr[(b, t0 + ti)] = ((pos + mn) // 128,
                                             (pos + mx - 1) // 128)
                else:
                    self.kr[(b, t0 + ti)] = None
            pos += glen
            if order == "bT" and T == NSUP - 1:
                self.blk_kt_base[b + 1] = pos // 128
        self.nktot = pos // 128
        self.ntok = pos


def _preprocess_v3(edge_index, cfg: CFG, order="bT"):
    """V3: edges only (no self-loop tokens), supertile-level padding.

    Returns (deg, Layout3, per_core)."""
    N, C, NBLK = cfg.N, cfg.C, cfg.NBLK
    NT, SHARD, BLK = cfg.NT, cfg.SHARD, cfg.BLK

    src = np.asarray(edge_index[0], dtype=np.int64)
    dst = np.asarray(edge_index[1], dtype=np.int64)
    loops_deg = np.concatenate([dst, np.arange(N, dtype=np.int64)])
    deg = np.bincount(loops_deg, minlength=N).astype(np.float32)

    core = dst // SHARD
    tloc = (dst % SHARD) // 128
    blk = src // BLK
    key = (core * NBLK + blk) * NT + tloc
    sort = np.argsort(key, kind="stable")
    s_src = src[sort]
    s_dst = dst[sort]

    ngroups = C * NBLK * NT
    cnt = np.bincount(key, minlength=ngroups).reshape(C, NBLK, NT)
    starts = np.zeros(ngroups + 1, dtype=np.int64)
    np.cumsum(cnt.reshape(-1), out=starts[1:])

    lay = Layout3(cfg, cnt, order=order, GT=(8 if order == "Tb" else 16))
    GT = lay.GT

    per_core = []
    for c in range(C):
        idx_stream = np.zeros(lay.ntok, dtype=np.int16)
        dloc_stream = np.full(lay.ntok, 9999.0, dtype=np.float32)
        for b in range(NBLK):
            for T in range(lay.NSUP):
                t0, t1 = T * GT, min((T + 1) * GT, NT)
                pos = int(lay.off_group[b, T])
                for t in range(t0, t1):
                    g = (c * NBLK + b) * NT + t
                    a, e = starts[g], starts[g + 1]
                    n = e - a
                    idx_stream[pos:pos + n] = (
                        s_src[a:e] - b * BLK).astype(np.int16)
                    dloc_stream[pos:pos + n] = (
                        s_dst[a:e] - (c * SHARD + T * GT * 128)
                    ).astype(np.float32)
                    pos += n
        idx_rep = np.ascontiguousarray(
            np.tile(idx_stream.reshape(-1, 16).T, (8, 1)))
        dloc_w = np.ascontiguousarray(
            dloc_stream.reshape(-1, 128).T).astype(np.float16)
        per_core.append({"idx": idx_rep, "dloc": dloc_w})

    return deg, lay, per_core


def _preprocess(edge_index, cfg: CFG):
    """Partition/sort edges, compute degrees, build per-core gather indices.

    Returns (deg, layout, per_core)."""
    N, C, NBLK = cfg.N, cfg.C, cfg.NBLK
    NT, SHARD, BLK = cfg.NT, cfg.SHARD, cfg.BLK

    loops = np.arange(N, dtype=np.int64)
    src = np.concatenate([np.asarray(edge_index[0], dtype=np.int64), loops])
    dst = np.concatenate([np.asarray(edge_index[1], dtype=np.int64), loops])
    deg = np.bincount(dst, minlength=N).astype(np.float32)

    core = dst // SHARD
    tloc = (dst % SHARD) // 128
    blk = src // BLK
    key = (core * NBLK + blk) * NT + tloc
    # secondary sort by src within each group: ascending gather addresses
    # give much better HBM locality for the 256B random reads
    order = np.argsort(key * BLK + (src - blk * BLK), kind="stable")
    s_src = src[order]
    s_dst = dst[order]

    ngroups = C * NBLK * NT
    cnt = np.bincount(key, minlength=ngroups).reshape(C, NBLK, NT)
    starts = np.zeros(ngroups + 1, dtype=np.int64)
    np.cumsum(cnt.reshape(-1), out=starts[1:])

    lay = Layout(cfg, cnt.max(axis=0))
    GT = lay.GT

    per_core = []
    for c in range(C):
        idx_stream = np.zeros(lay.ntok, dtype=np.int16)
        dloc_stream = np.full(lay.ntok, 9999.0, dtype=np.float32)
        for b in range(NBLK):
            for t in range(NT):
                g = (c * NBLK + b) * NT + t
                a, e = starts[g], starts[g + 1]
                n = e - a
                pos = lay.off[b, t]
                idx_stream[pos:pos + n] = (s_src[a:e] - b * BLK).astype(np.int16)
                dloc_stream[pos:pos + n] = (
                    s_dst[a:e] - (c * SHARD + (t // GT) * GT * 128)
                ).astype(np.float32)
        idx_rep = np.ascontiguousarray(
            np.tile(idx_stream.reshape(-1, 16).T, (8, 1))
        )  # [128, ntok//16]
        dloc_w = np.ascontiguousarray(
            dloc_stream.reshape(-1, 128).T
        ).astype(np.float16)  # [128, nktot]
        per_core.append({"idx": idx_rep, "dloc": dloc_w})

    return deg, lay, per_core


def _wrap_deg(deg, cfg: CFG):
    """degB [128, NBLK*NBT] (block-wrapped, pad 1.0) and per-core degS
    [128, NT] (shard-wrapped, pad 1.0)."""
    N, NBLK, BLK, NBT = cfg.N, cfg.NBLK, cfg.BLK, cfg.NBT
    C, SHARD, NT = cfg.C, cfg.SHARD, cfg.NT
    degB = np.ones((128, NBLK * NBT), dtype=np.float32)
    for b in range(NBLK):
        for j in range(NBT):
            base = b * BLK + j * 128
            m = min(128, (b + 1) * BLK - base, N - base)
            if m > 0:
                degB[:m, b * NBT + j] = deg[base:base + m]
    degS = np.ones((C, 128, NT), dtype=np.float32)
    for c in range(C):
        for t in range(NT):
            base = c * SHARD + t * 128
            m = min(128, (c + 1) * SHARD - base)
            degS[c, :m, t] = deg[base:base + m]
    return degB, degS


# ------------------------------------------------------------ bass program --
def _build(cfg: CFG, lay: Layout, ablate=()):
    import concourse.bacc as bacc
    import concourse.mybir as mybir
    from concourse import tile

    f32 = mybir.dt.float32
    f16 = mybir.dt.float16
    i16 = mybir.dt.int16
    i32 = mybir.dt.int32
    ALU = mybir.AluOpType
    ACTF = mybir.ActivationFunctionType

    N, F, H, O, C = cfg.N, cfg.F, cfg.H, cfg.O, cfg.C
    NBLK, BLK, NBT = cfg.NBLK, cfg.BLK, cfg.NBT
    NT, SHARD, LASTV, HP = cfg.NT, cfg.SHARD, cfg.LASTV, cfg.HP
    CH_KT, SLAB, STG = cfg.CH_KT, cfg.SLAB, cfg.STG

    nktot = lay.nktot
    ntok = lay.ntok
    blk_kt_base = lay.blk_kt_base
    GT = lay.GT
    v3 = isinstance(lay, Layout3)

    nc = bacc.Bacc("TRN2", target_bir_lowering=False, debug=False,
                   num_devices=cfg.C,
                   dynamic_dma_scratch_size=cfg.DMA_SCRATCH,
                   num_swdge_queues=4)

    # ---- DRAM I/O
    shard_build_pre = v3 and "oldbuild" not in ablate
    xT_d = (None if shard_build_pre else
            nc.dram_tensor("xT", [F, N], f32, kind="ExternalInput"))
    xTs_d = (nc.dram_tensor("xTs", [F, SHARD], f32, kind="ExternalInput")
             if v3 else None)
    degB_d = (None if shard_build_pre else
              nc.dram_tensor("degB", [128, NBLK * NBT], f32,
                             kind="ExternalInput"))
    degS_d = nc.dram_tensor("degS", [128, NT], f32, kind="ExternalInput")
    idx_d = nc.dram_tensor("idx", [128, ntok // 16], i16, kind="ExternalInput")
    dloc_d = nc.dram_tensor("dloc", [128, nktot], f16, kind="ExternalInput")
    W1_d = nc.dram_tensor("W1", [F, H], f32, kind="ExternalInput")
    W2_d = nc.dram_tensor("W2", [H, O], f32, kind="ExternalInput")
    b1_d = nc.dram_tensor("b1", [1, H], f32, kind="ExternalInput")
    b2_d = nc.dram_tensor("b2", [1, O], f32, kind="ExternalInput")
    P_d = nc.dram_tensor("P", [1, O], f32, kind="ExternalInput")
    K_d = nc.dram_tensor("K", [1, O], f32, kind="ExternalInput")
    U_d = nc.dram_tensor("U", [1, O], f32, kind="ExternalInput")
    out_d = nc.dram_tensor("out", [SHARD, O], f32, kind="ExternalOutput")

    TW = 128  # f16 table row width (256B gather granule; cols >= H unused)
    TWv = 2 * TW if "elem512" in ablate else TW
    shard_build = v3 and "oldbuild" not in ablate
    if shard_build:
        t1loc = nc.dram_tensor("t1loc", [SHARD, TWv], f16)
        tab1f = nc.dram_tensor("tab1f", [N, TWv], f16, addr_space="Shared")
        tab1 = [tab1f[b * BLK: b * BLK + min(BLK, N - b * BLK), :]
                for b in range(NBLK)]
    else:
        tab1 = [
            nc.dram_tensor(f"tab1_{b}", [min(BLK, N - b * BLK), TWv], f16)
            for b in range(NBLK)
        ]
    t2loc = nc.dram_tensor("t2loc", [SHARD, TWv], f16)
    tab2 = nc.dram_tensor("tab2", [N, TWv], f16, addr_space="Shared")

    with tile.TileContext(nc, num_cores=C) as tc:
        with (
            tc.tile_pool(name="const", bufs=1) as const,
            tc.tile_pool(name="xslab", bufs=2) as xpool,
            tc.tile_pool(name="t1st", bufs=2) as t1pool,
            tc.tile_pool(name="g", bufs=16) as gpool,
            tc.tile_pool(name="s", bufs=6) as spool,
            tc.tile_pool(name="work", bufs=2) as work,
            tc.tile_pool(name="post", bufs=2) as post,
            tc.tile_pool(name="ost", bufs=2) as opool,
            tc.tile_pool(name="ps_seg", bufs=3, space="PSUM") as ps_seg,
            tc.tile_pool(name="ps_bld", bufs=1, space="PSUM") as ps_bld,
            tc.tile_pool(name="ps_tr", bufs=2, space="PSUM") as ps_tr,
            tc.tile_pool(name="ps_t2", bufs=2, space="PSUM") as ps_t2,
        ):
            # ---------------- constants / small inputs
            iota_i = const.tile([128, GT * 128], i32)
            nc.gpsimd.iota(iota_i[:, :], pattern=[[128, GT], [1, 128]],
                           base=0, channel_multiplier=0)
            IOTA16 = const.tile([128, GT * 128], f16)
            nc.vector.tensor_copy(IOTA16[:, :], iota_i[:, :])
            IOTA = IOTA16  # first 128 columns are a plain 0..127 iota row
            IDiota = const.tile([128, 128], f32)
            pidx_i = const.tile([128, 1], i32)
            nc.gpsimd.iota(pidx_i[:, :], pattern=[[0, 1]], base=0,
                           channel_multiplier=1)
            PIDX = const.tile([128, 1], f32)
            nc.vector.tensor_copy(PIDX[:, :], pidx_i[:, :])
            ID = const.tile([128, 128], f32)
            nc.vector.tensor_copy(IDiota[:, :], iota_i[:, :128])
            nc.vector.tensor_scalar(out=ID[:, :], in0=IDiota[:, :],
                                    scalar1=PIDX[:, :], scalar2=None,
                                    op0=ALU.is_equal)

            W1s = const.tile([F, H], f32)
            nc.sync.dma_start(W1s[:, :], W1_d[:, :])
            W2s = const.tile([H, O], f32)
            nc.sync.dma_start(W2s[:, :], W2_d[:, :])

            b1row = const.tile([1, H], f32)
            nc.sync.dma_start(b1row[:, :], b1_d[:, :])
            BIAS1 = const.tile([128, H], f32)
            nc.gpsimd.partition_broadcast(BIAS1[:, :], b1row[:, :])

            b2row = const.tile([1, O], f32)
            nc.sync.dma_start(b2row[:, :], b2_d[:, :])
            prow = const.tile([1, O], f32)
            nc.sync.dma_start(prow[:, :], P_d[:, :])
            krow = const.tile([1, O], f32)
            nc.sync.dma_start(krow[:, :], K_d[:, :])
            urow = const.tile([1, O], f32)
            nc.sync.dma_start(urow[:, :], U_d[:, :])
            pku = const.tile([1, O], f32)
            nc.vector.tensor_mul(pku[:, :], prow[:, :], krow[:, :])
            nc.vector.tensor_mul(pku[:, :], pku[:, :], urow[:, :])
            nc.vector.tensor_add(pku[:, :], pku[:, :], b2row[:, :])
            BIAS2 = const.tile([128, O], f32)
            nc.gpsimd.partition_broadcast(BIAS2[:, :], pku[:, :])

            if degB_d is not None:
                degB = const.tile([128, NBLK * NBT], f32)
                nc.sync.dma_start(degB[:, :], degB_d[:, :])
                dinvB = const.tile([128, NBLK * NBT], f32)
                nc.vector.reciprocal(dinvB[:, :], degB[:, :])
                nc.scalar.sqrt(dinvB[:, :], dinvB[:, :])

            degS = const.tile([128, NT], f32)
            nc.sync.dma_start(degS[:, :], degS_d[:, :])
            dinvS = const.tile([128, NT], f32)
            nc.vector.reciprocal(dinvS[:, :], degS[:, :])
            nc.scalar.sqrt(dinvS[:, :], dinvS[:, :])

            idxS = const.tile([128, ntok // 16], i16)
            nc.sync.dma_start(idxS[:, :], idx_d[:, :])
            dloc = const.tile([128, nktot], f16)
            nc.sync.dma_start(dloc[:, :], dloc_d[:, :])

            agg = const.tile([128, NT, H], f32)
            if v3:
                # Own-shard pass over dinv*(x@W1): initializes agg with the
                # self-loop table rows AND (shard_build) stages this core's
                # slice of the layer-1 table for the AllGather.
                for s0 in range(0, SHARD, SLAB):
                    w = min(SLAB, SHARD - s0)
                    xs = xpool.tile([F, SLAB], f32, tag="xss")
                    nc.sync.dma_start(xs[:, :w], xTs_d[:, s0:s0 + w])
                    st = None
                    if shard_build:
                        st = t1pool.tile([128, _cdiv(SLAB, 128), H], f16,
                                         tag="t1st")
                    nfull = 0
                    for j0 in range(0, w, 128):
                        m = min(128, w - j0)
                        t = (s0 + j0) // 128
                        ps = ps_bld.tile([128, H], f32, tag="psb")
                        if m < 128:
                            # pad rows: zero so downstream stays finite
                            nc.vector.memset(agg[:, t, :], 0.0)
                        nc.tensor.matmul(ps[:m, :], lhsT=xs[:, j0:j0 + m],
                                         rhs=W1s[:, :], start=True, stop=True)
                        nc.scalar.activation(
                            agg[:m, t, :], ps[:m, :], ACTF.Copy,
                            scale=dinvS[:m, t:t + 1])
                        if shard_build:
                            nc.scalar.activation(
                                st[:m, j0 // 128, :], ps[:m, :], ACTF.Copy,
                                scale=dinvS[:m, t:t + 1])
                            if m == 128:
                                nfull += 1
                    if shard_build:
                        if nfull:
                            dst_ap = t1loc[s0:s0 + nfull * 128, :H].rearrange(
                                "(j p) f -> p j f", p=128)
                            nc.sync.dma_start(dst_ap, st[:, :nfull, :])
                        if nfull * 128 < w:
                            mm = w - nfull * 128
                            nc.sync.dma_start(
                                t1loc[s0 + nfull * 128: s0 + w, :H],
                                st[:mm, nfull, :])
                if shard_build:
                    nc.gpsimd.collective_compute(
                        "AllGather", mybir.AluOpType.bypass,
                        replica_groups=[list(range(C))],
                        ins=[t1loc[:, :].opt()],
                        outs=[tab1f[:, :].opt()])
            else:
                nc.vector.memset(agg[:, :, :], 0.0)

            # ---------------- layer-1 message table: tab1_b = dinv*(x@W1)
            def build_table1(b):
                nodes_b = min(BLK, N - b * BLK)
                for s0 in range(0, nodes_b, SLAB):
                    w = min(SLAB, nodes_b - s0)
                    xs = xpool.tile([F, SLAB], f32, tag="xs")
                    nc.sync.dma_start(xs[:, :w],
                                      xT_d[:, b * BLK + s0: b * BLK + s0 + w])
                    st = t1pool.tile([128, _cdiv(SLAB, 128), H], f16, tag="t1st")
                    nfull = 0
                    for j0 in range(0, w, 128):
                        m = min(128, w - j0)
                        jt = (s0 + j0) // 128  # node-tile idx within block
                        ps = ps_bld.tile([128, H], f32, tag="psb")
                        nc.tensor.matmul(ps[:m, :], lhsT=xs[:, j0:j0 + m],
                                         rhs=W1s[:, :], start=True, stop=True)
                        nc.scalar.activation(
                            st[:m, j0 // 128, :], ps[:m, :], ACTF.Copy,
                            scale=dinvB[:m, b * NBT + jt: b * NBT + jt + 1])
                        if m == 128:
                            nfull += 1
                    # store staged tiles to DRAM
                    if nfull:
                        dst_ap = tab1[b][s0:s0 + nfull * 128, :H].rearrange(
                            "(j p) f -> p j f", p=128)
                        nc.sync.dma_start(dst_ap, st[:, :nfull, :])
                    if nfull * 128 < w:  # ragged tail tile of the block
                        m = w - nfull * 128
                        nc.sync.dma_start(
                            tab1[b][s0 + nfull * 128: s0 + w, :H],
                            st[:m, nfull, :])

            if "nobuild" not in ablate and not shard_build:
                for b in range(NBLK):
                    build_table1(b)

            # ---------------- gather + segment-sum matmul for one layer
            MAXKB = 8  # S-matrices built per DVE instruction
            qrot = [0]  # SWDGE queue rotation across gather calls

            def seg_layer(table_aps, uw):
                """table_aps[b]: block b's [rows, TW] f16 message rows; only
                the first uw columns are meaningful."""
                for b in range(NBLK):
                    kt_in_blk = blk_kt_base[b + 1] - blk_kt_base[b]
                    if kt_in_blk == 0:
                        continue
                    # gather chunks
                    gtiles = []
                    SP = "singlepacket" in ablate
                    for ci in range(_cdiv(kt_in_blk, CH_KT)):
                        kts = min(CH_KT, kt_in_blk - ci * CH_KT)
                        g = gpool.tile([128, CH_KT, TWv], f16, tag="g")
                        tok0 = (blk_kt_base[b] + ci * CH_KT) * 128
                        if "smallgather" in ablate:
                            nc.gpsimd.dma_gather(
                                g[:, :1, :], table_aps[b],
                                idxS[:, tok0 // 16: (tok0 + 128) // 16],
                                num_idxs=128, num_idxs_reg=128,
                                elem_size=TWv, single_packet=SP,
                                queue_num=qrot[0] % 4)
                        elif "smallreg" in ablate:
                            nc.gpsimd.dma_gather(
                                g[:, :kts, :], table_aps[b],
                                idxS[:, tok0 // 16: (tok0 + kts * 128) // 16],
                                num_idxs=kts * 128, num_idxs_reg=128,
                                elem_size=TWv, single_packet=SP,
                                queue_num=qrot[0] % 4)
                        else:
                            nc.gpsimd.dma_gather(
                                g[:, :kts, :], table_aps[b],
                                idxS[:, tok0 // 16: (tok0 + kts * 128) // 16],
                                num_idxs=kts * 128, num_idxs_reg=kts * 128,
                                elem_size=TWv, single_packet=SP,
                                queue_num=qrot[0] % 4)
                        qrot[0] += 1
                        gtiles.append(g)

                    def gslice(kglob):
                        ci, sl = divmod(kglob - blk_kt_base[b], CH_KT)
                        return gtiles[ci][:, sl, :uw]

                    # consume: per dst-tile, its token range [o0, o1) in the
                    # common layout; K-tiles at supertile boundaries are
                    # shared between adjacent dst-tiles (S masks the others).
                    if "noconsume" in ablate:
                        continue
                    for t in range(NT):
                        if v3:
                            r = lay.kr[(b, t)]
                            if r is None:
                                continue
                            k0, k1 = r
                        else:
                            ct = int(lay.ctok[b, t])
                            if ct == 0:
                                continue
                            o0 = int(lay.off[b, t])
                            o1 = o0 + ct
                            k0, k1 = o0 // 128, (o1 - 1) // 128
                        it = t % GT  # iota variant within supertile
                        ps = ps_seg.tile([128, uw], f32, tag="pss")
                        k = k0
                        while k <= k1:
                            kb = min(MAXKB, k1 + 1 - k)
                            Sb = spool.tile([128, MAXKB, 128], f16, tag="s")
                            nc.vector.tensor_tensor(
                                out=Sb[:, :kb, :],
                                in0=IOTA16[:, it * 128:(it + 1) * 128]
                                    .unsqueeze(1)
                                    .broadcast_to([128, kb, 128]),
                                in1=dloc[:, k:k + kb].unsqueeze(2)
                                    .broadcast_to([128, kb, 128]),
                                op=ALU.is_equal)
                            for j in range(kb):
                                nc.tensor.matmul(
                                    ps[:, :], lhsT=Sb[:, j, :],
                                    rhs=gslice(k + j),
                                    start=(k + j == k0),
                                    stop=(k + j == k1))
                            k += kb
                        nc.vector.tensor_add(agg[:, t, :uw],
                                             agg[:, t, :uw], ps[:, :])

            def seg_layer_tb(table_aps, uw):
                """order='Tb': per supertile, gather all 4 block groups,
                then one PSUM accumulation per dst-tile across blocks."""
                SP = "singlepacket" in ablate
                for T in range(lay.NSUP):
                    gmap = {}
                    for b in range(NBLK):
                        off = int(lay.off_group[b, T])
                        kt_grp = int(lay.glen[b, T]) // 128
                        k_base = off // 128
                        ci = 0
                        while ci < kt_grp:
                            kts = min(CH_KT, kt_grp - ci)
                            g = gpool.tile([128, CH_KT, TWv], f16, tag="g")
                            tok0 = (k_base + ci) * 128
                            nc.gpsimd.dma_gather(
                                g[:, :kts, :], table_aps[b],
                                idxS[:, tok0 // 16:
                                     (tok0 + kts * 128) // 16],
                                num_idxs=kts * 128, num_idxs_reg=kts * 128,
                                elem_size=TWv, single_packet=SP,
                                queue_num=qrot[0] % 4)
                            qrot[0] += 1
                            for s in range(kts):
                                gmap[k_base + ci + s] = (g, s)
                            ci += kts
                    if "noconsume" in ablate:
                        continue
                    t0, t1 = T * GT, min((T + 1) * GT, NT)
                    for t in range(t0, t1):
                        spans = [lay.kr[(b, t)] for b in range(NBLK)
                                 if lay.kr[(b, t)] is not None]
                        if not spans:
                            continue
                        it = t % GT
                        firstk = spans[0][0]
                        lastk = spans[-1][1]
                        ps = ps_seg.tile([128, uw], f32, tag="pss")
                        for (k0, k1) in spans:
                            k = k0
                            while k <= k1:
                                kb = min(MAXKB, k1 + 1 - k)
                                Sb = spool.tile([128, MAXKB, 128], f16,
                                                tag="s")
                                nc.vector.tensor_tensor(
                                    out=Sb[:, :kb, :],
                                    in0=IOTA16[:, it * 128:(it + 1) * 128]
                                        .unsqueeze(1)
                                        .broadcast_to([128, kb, 128]),
                                    in1=dloc[:, k:k + kb].unsqueeze(2)
                                        .broadcast_to([128, kb, 128]),
                                    op=ALU.is_equal)
                                for j in range(kb):
                                    gt_, sl = gmap[k + j]
                                    nc.tensor.matmul(
                                        ps[:, :], lhsT=Sb[:, j, :],
                                        rhs=gt_[:, sl, :uw],
                                        start=(k + j == firstk),
                                        stop=(k + j == lastk))
                                k += kb
                        nc.vector.tensor_add(agg[:, t, :uw],
                                             agg[:, t, :uw], ps[:, :])

            seg = seg_layer_tb if (v3 and lay.order == "Tb") else seg_layer

            # ---------------- layer 1
            tab1_aps = (tab1 if shard_build
                        else [tab1[b][:, :] for b in range(NBLK)])
            seg(tab1_aps, H)

            # post: h1 = relu(dinv*agg + b1); t2 = dinv*(h1@W2) padded
            def staged_store(dram, stile, grp, nt_in_grp, width):
                """store staging tile rows [grp*STG .. ) handling ragged tail"""
                t0 = grp * STG
                nfull = 0
                for tt in range(nt_in_grp):
                    if (t0 + tt) * 128 + 128 <= SHARD:
                        nfull += 1
                if nfull:
                    dst = dram[t0 * 128: t0 * 128 + nfull * 128,
                               :width].rearrange("(j p) f -> p j f", p=128)
                    nc.sync.dma_start(dst, stile[:, :nfull, :width])
                if nfull < nt_in_grp:
                    nc.sync.dma_start(
                        dram[(t0 + nfull) * 128: SHARD, :width],
                        stile[:LASTV, nfull, :width])

            for grp in range(_cdiv(NT, STG)):
                nt_in_grp = min(STG, NT - grp * STG)
                st = post.tile([128, STG, H], f16, tag="t2st")
                # cols O:H of t2loc/tab2 are never read (the L2 consume
                # matmul rhs spans only :O), so no zeroing is needed
                for tt in range(nt_in_grp):
                    t = grp * STG + tt
                    h1 = work.tile([128, H], f32, tag="h1")
                    nc.vector.scalar_tensor_tensor(
                        out=h1[:, :], in0=agg[:, t, :],
                        scalar=dinvS[:, t:t + 1], in1=BIAS1[:, :],
                        op0=ALU.mult, op1=ALU.add)
                    nc.scalar.activation(h1[:, :], h1[:, :], ACTF.Relu)
                    pst = ps_tr.tile([H, 128], f32, tag="pstr")
                    nc.tensor.transpose(pst[:, :], h1[:, :], ID[:, :])
                    h1t = work.tile([H, 128], f32, tag="h1t")
                    nc.scalar.copy(h1t[:, :], pst[:, :])
                    ps2 = ps_t2.tile([128, O], f32, tag="pst2")
                    nc.tensor.matmul(ps2[:, :], lhsT=h1t[:, :], rhs=W2s[:, :],
                                     start=True, stop=True)
                    nc.scalar.activation(st[:, tt, :O], ps2[:, :], ACTF.Copy,
                                         scale=dinvS[:, t:t + 1])
                    if v3:
                        # L2 self-loop init: agg[:, t, :O] = dinv*(h1@W2)
                        # (own t2 table row; postproc applies dst-side dinv)
                        nc.scalar.activation(
                            agg[:, t, :O], ps2[:, :], ACTF.Copy,
                            scale=dinvS[:, t:t + 1])
                staged_store(t2loc, st, grp, nt_in_grp, H)

            # ---------------- exchange layer-2 table
            nc.gpsimd.collective_compute(
                "AllGather", mybir.AluOpType.bypass,
                replica_groups=[list(range(C))],
                ins=[t2loc[:, :].opt()],
                outs=[tab2[:, :].opt()])
            if "agx2" in ablate:  # probe: cost of one extra AllGather
                nc.gpsimd.collective_compute(
                    "AllGather", mybir.AluOpType.bypass,
                    replica_groups=[list(range(C))],
                    ins=[t2loc[:, :].opt()],
                    outs=[tab2[:, :].opt()])

            # ---------------- layer 2
            if not v3:
                nc.vector.memset(agg[:, :, :], 0.0)
            tab2_aps = [tab2[b * BLK: b * BLK + min(BLK, N - b * BLK), :]
                        for b in range(NBLK)]
            seg(tab2_aps, O)

            # post: y = relu(dinv*agg + b2 + pku); out = log_softmax(y)
            for grp in range(0 if "nopost2" not in ablate
                             else _cdiv(NT, STG), _cdiv(NT, STG)):
                nt_in_grp = min(STG, NT - grp * STG)
                st = opool.tile([128, STG, O], f32, tag="ost")
                for tt in range(nt_in_grp):
                    t = grp * STG + tt
                    y = work.tile([128, O], f32, tag="y")
                    nc.vector.scalar_tensor_tensor(
                        out=y[:, :], in0=agg[:, t, :O],
                        scalar=dinvS[:, t:t + 1], in1=BIAS2[:, :],
                        op0=ALU.mult, op1=ALU.add)
                    nc.scalar.activation(y[:, :], y[:, :], ACTF.Relu)
                    nmax = work.tile([128, 1], f32, tag="nmax")
                    nc.vector.tensor_reduce(nmax[:, :], y[:, :],
                                            axis=mybir.AxisListType.X,
                                            op=ALU.max, negate=True)
                    ex = work.tile([128, O], f32, tag="ex")
                    esum = work.tile([128, 1], f32, tag="esum")
                    nc.scalar.activation(ex[:, :], y[:, :], ACTF.Exp,
                                         bias=nmax[:, :], scale=1.0,
                                         accum_out=esum[:, :])
                    lsum = work.tile([128, 1], f32, tag="lsum")
                    nc.scalar.activation(lsum[:, :], esum[:, :], ACTF.Ln)
                    nc.vector.tensor_scalar(
                        out=st[:, tt, :], in0=y[:, :], scalar1=nmax[:, :],
                        scalar2=lsum[:, :], op0=ALU.add, op1=ALU.subtract)
                staged_store(out_d, st, grp, nt_in_grp, O)

    nc.compile()
    return nc


# ------------------------------------------------------------------ entry --
def make_in_maps(inputs, cfg, per_core, degB, degS, xT, v3):
    in_maps = []
    for c in range(cfg.C):
        m = {
            "xT": xT,
            "degB": degB,
            "degS": np.ascontiguousarray(degS[c]),
            "idx": per_core[c]["idx"],
            "dloc": per_core[c]["dloc"],
            "W1": np.asarray(inputs["W1"], np.float32),
            "W2": np.asarray(inputs["W2"], np.float32),
            "b1": np.asarray(inputs["b1"], np.float32).reshape(1, -1),
            "b2": np.asarray(inputs["b2"], np.float32).reshape(1, -1),
            "P": np.asarray(inputs["P"], np.float32).reshape(1, -1),
            "K": np.asarray(inputs["K"], np.float32).reshape(1, -1),
            "U": np.asarray(inputs["U"], np.float32).reshape(1, -1),
        }
        if v3:
            m["xTs"] = np.ascontiguousarray(
                xT[:, c * cfg.SHARD:(c + 1) * cfg.SHARD])
        in_maps.append(m)
    return in_maps


def prepare_and_run(inputs, cfg=None, trace=False, v3=True, **run_kwargs):
    """Preprocess, build, run on 8 cores.  Returns (out, BassKernelResults)."""
    from concourse.bass_utils import run_bass_kernel_spmd

    cfg = cfg or CFG()
    x = np.asarray(inputs["x"], dtype=np.float32)
    edge_index = np.asarray(inputs["edge_index"])

    pre = _preprocess_v3 if v3 else _preprocess
    deg, lay, per_core = pre(edge_index, cfg)
    degB, degS = _wrap_deg(deg, cfg)
    xT = np.ascontiguousarray(x.T)

    nc = _build(cfg, lay)

    in_maps = make_in_maps(inputs, cfg, per_core, degB, degS, xT, v3)
    res = run_bass_kernel_spmd(nc, in_maps, core_ids=list(range(cfg.C)),
                               trace=trace, **run_kwargs)
    out = np.concatenate([res.results[c]["out"] for c in range(cfg.C)], axis=0)
    return out.astype(np.float32), res


def kernel(**inputs):
    out, _ = prepare_and_run(inputs)
    return out


if __name__ == "__main__":
    import reference

    inputs = {k: np.asarray(v) for k, v in reference.setup_inputs().items()}
    got = kernel(**inputs)
    want = np.asarray(reference.reference(**inputs))
    err = np.abs(got - want).max() / max(np.abs(want).max(), 1e-9)
    print("rel err:", err)



# revision 50
# speedup vs baseline: 1.3673x; 1.1648x over previous
"""Bass/Trainium2 kernel for a 2-layer GCN with knowledge-enhanced output
(nn_KeGNN): y = log_softmax(relu(GCN2(relu(GCN1(x))) + P*K*U)).

Distribution strategy (8 NeuronCores, SPMD one NEFF):
  * Nodes are partitioned into 8 contiguous shards (12500 each); core c owns
    the edges whose *destination* is in shard c and produces the output rows
    of its shard.
  * GCN normalization is folded node-wise: with dinv = 1/sqrt(deg),
    table = dinv * (H @ W) gives messages, and the aggregated sum is scaled
    by dinv[dst].  The per-edge segment-sum becomes:
       agg[dst-tile] += S.T @ G        (TensorE matmul, PSUM accumulate)
    where G = dma_gather(table, src-index) and S is a 0/1 selection matrix
    built on VectorE with one is_equal against a static iota row.
  * Both layer tables are built per-shard and AllGathered (cheap, ~80us):
    layer 1 from a per-core [F, SHARD] slice of x^T, layer 2 from h1@W2 in
    the layer-1 postproc.  The same own-shard matmul pass also initializes
    agg with the self-loop contribution (dinv * own table row), so explicit
    self-loop gather tokens are not needed.
  * Source indices are int16 (hardware gather limit 32767) so the gather is
    split into 4 source blocks of 25000 nodes.  Edge tokens are grouped by
    (src-block, dst-supertile of 16 tiles) and padded to the cross-core max
    only at group granularity; each dst-tile consumes the cross-core
    min/max K-tile span of its group, with the S masks (built from per-core
    dstloc data) zeroing other cores' overhang.  One program serves all 8
    cores; per-core behavior differs only through input data.
  * Gather descriptors cost ~1.3ns each on 4 SWDGE queues (the dominant
    serial term together with DVE instruction count), so the layout
    minimizes token count: 205k tokens/core/layer vs 212.5k edges+loops.
"""

import numpy as np


# ----------------------------------------------------------------- config --
class CFG:
    N = 100000      # nodes
    F = 128         # input feature dim
    H = 64          # hidden dim
    O = 40          # output dim
    E = 1600000     # edges (without self loops)
    C = 8           # cores
    NBLK = 4        # src blocks (int16 gather index limit)
    CH_KT = 16      # K-tiles (of 128 tokens) per dma_gather call
                    # (2048 descriptors/call; needs DMA_SCRATCH >= 32KB)
    SLAB = 2048     # nodes per xT slab load in table1 build
    DMA_SCRATCH = 32768   # per-partition SWDGE desc-ring carveout bytes
    STG = 25        # dst-tiles per staged DRAM write in postproc

    def __init__(self, **kw):
        for k, v in kw.items():
            setattr(self, k, v)
        assert self.N % self.C == 0
        self.SHARD = self.N // self.C
        self.NT = -(-self.SHARD // 128)          # dst tiles per core
        self.LASTV = self.SHARD - (self.NT - 1) * 128  # valid rows in last tile
        assert self.N % self.NBLK == 0
        self.BLK = self.N // self.NBLK
        assert self.BLK <= 32767
        self.NBT = -(-self.BLK // 128)           # node tiles per block
        self.HP = 64                             # padded layer-2 table width
        assert self.O <= self.HP


def _cdiv(a, b):
    return -(-a // b)


# ----------------------------------------------------- host preprocessing --
class Layout:
    """Cross-core-common token layout.

    Tokens are grouped by (src-block b, dst-supertile T, dst-tile t); each
    (b, t) group gets the cross-core max token count (ctok), supertile
    streams are padded to multiples of 128 so K-tiles never span supertiles.
    dstloc values are relative to the supertile base (< GT*128).
    """

    GT = 16  # dst tiles per supertile

    def __init__(self, cfg: CFG, ctok):
        self.ctok = ctok  # [NBLK, NT] common per-(b,t) token counts
        NT, NBLK = cfg.NT, cfg.NBLK
        self.NSUP = _cdiv(NT, self.GT)
        self.off = np.zeros((NBLK, NT), dtype=np.int64)  # global token offset
        self.nk_sup = np.zeros((NBLK, self.NSUP), dtype=np.int64)
        self.blk_kt_base = [0] * (NBLK + 1)
        pos = 0
        for b in range(NBLK):
            for T in range(self.NSUP):
                t0, t1 = T * self.GT, min((T + 1) * self.GT, NT)
                sup_len = 0
                for t in range(t0, t1):
                    self.off[b, t] = pos + sup_len
                    sup_len += int(ctok[b, t])
                sup_pad = _cdiv(sup_len, 128) * 128
                self.nk_sup[b, T] = sup_pad // 128
                pos += sup_pad
            self.blk_kt_base[b + 1] = pos // 128
        self.nktot = pos // 128
        self.ntok = pos


class Layout3:
    """V3 token layout: edge tokens only (self-loops handled densely),
    grouped by (src-block b, dst-supertile T) with padding at group level;
    per dst-tile K-ranges are the cross-core min/max span (S masks the
    out-of-range tokens of other cores)."""

    def __init__(self, cfg: CFG, cnt, order="bT", GT=16):
        # cnt: [C, NBLK, NT] per-core per-(block, dst-tile) edge counts
        # order "bT": groups laid out block-major (gathers per block,
        #             consume per (b, t), 4 agg adds per tile).
        # order "Tb": supertile-major (per tile, one PSUM accumulation
        #             across all 4 blocks, single agg add).
        C, NBLK, NT = cfg.C, cfg.NBLK, cfg.NT
        self.GT = GT
        NSUP = _cdiv(NT, GT)
        self.NSUP = NSUP
        self.order = order
        self.off_group = np.zeros((NBLK, NSUP), dtype=np.int64)
        self.glen = np.zeros((NBLK, NSUP), dtype=np.int64)
        self.kr = {}          # (b, t) -> (k0, k1) global K-tile span or None
        self.blk_kt_base = [0] * (NBLK + 1)
        if order == "bT":
            pairs = [(b, T) for b in range(NBLK) for T in range(NSUP)]
        else:
            pairs = [(b, T) for T in range(NSUP) for b in range(NBLK)]
        pos = 0
        for b, T in pairs:
            t0, t1 = T * GT, min((T + 1) * GT, NT)
            g = cnt[:, b, t0:t1]                      # [C, tiles]
            pre = np.concatenate(
                [np.zeros((C, 1), np.int64), np.cumsum(g, axis=1)],
                axis=1)
            gmax = int(pre[:, -1].max())
            glen = _cdiv(gmax, 128) * 128
            self.off_group[b, T] = pos
            self.glen[b, T] = glen
            for ti in range(t1 - t0):
                mn = int(pre[:, ti].min())
                mx = int(pre[:, ti + 1].max())
                if mx > mn:
                    self.kr[(b, t0 + ti)] = ((pos + mn) // 128,
                                             (pos + mx - 1) // 128)
                else:
                    self.kr[(b, t0 + ti)] = None
            pos += glen
            if order == "bT" and T == NSUP - 1:
                self.blk_kt_base[b + 1] = pos // 128
        self.nktot = pos // 128
        self.ntok = pos


def _preprocess_v3(edge_index, cfg: CFG, order="bT"):
    """V3: edges only (no self-loop tokens), supertile-level padding.

    Returns (deg, Layout3, per_core)."""
    N, C, NBLK = cfg.N, cfg.C, cfg.NBLK
    NT, SHARD, BLK = cfg.NT, cfg.SHARD, cfg.BLK

    src = np.asarray(edge_index[0], dtype=np.int64)
    dst = np.asarray(edge_index[1], dtype=np.int64)
    loops_deg = np.concatenate([dst, np.arange(N, dtype=np.int64)])
    deg = np.bincount(loops_deg, minlength=N).astype(np.float32)

    core = dst // SHARD
    tloc = (dst % SHARD) // 128
    blk = src // BLK
    key = (core * NBLK + blk) * NT + tloc
    sort = np.argsort(key, kind="stable")
    s_src = src[sort]
    s_dst = dst[sort]

    ngroups = C * NBLK * NT
    cnt = np.bincount(key, minlength=ngroups).reshape(C, NBLK, NT)
    starts = np.zeros(ngroups + 1, dtype=np.int64)
    np.cumsum(cnt.reshape(-1), out=starts[1:])

    lay = Layout3(cfg, cnt, order=order, GT=(8 if order == "Tb" else 16))
    GT = lay.GT

    per_core = []
    for c in range(C):
        idx_stream = np.zeros(lay.ntok, dtype=np.int16)
        dloc_stream = np.full(lay.ntok, 9999.0, dtype=np.float32)
        for b in range(NBLK):
            for T in range(lay.NSUP):
                t0, t1 = T * GT, min((T + 1) * GT, NT)
                pos = int(lay.off_group[b, T])
                for t in range(t0, t1):
                    g = (c * NBLK + b) * NT + t
                    a, e = starts[g], starts[g + 1]
                    n = e - a
                    idx_stream[pos:pos + n] = (
                        s_src[a:e] - b * BLK).astype(np.int16)
                    dloc_stream[pos:pos + n] = (
                        s_dst[a:e] - (c * SHARD + T * GT * 128)
                    ).astype(np.float32)
                    pos += n
        idx_rep = np.ascontiguousarray(
            np.tile(idx_stream.reshape(-1, 16).T, (8, 1)))
        dloc_w = np.ascontiguousarray(
            dloc_stream.reshape(-1, 128).T).astype(np.float16)
        per_core.append({"idx": idx_rep, "dloc": dloc_w})

    return deg, lay, per_core


def _preprocess(edge_index, cfg: CFG):
    """Partition/sort edges, compute degrees, build per-core gather indices.

    Returns (deg, layout, per_core)."""
    N, C, NBLK = cfg.N, cfg.C, cfg.NBLK
    NT, SHARD, BLK = cfg.NT, cfg.SHARD, cfg.BLK

    loops = np.arange(N, dtype=np.int64)
    src = np.concatenate([np.asarray(edge_index[0], dtype=np.int64), loops])
    dst = np.concatenate([np.asarray(edge_index[1], dtype=np.int64), loops])
    deg = np.bincount(dst, minlength=N).astype(np.float32)

    core = dst // SHARD
    tloc = (dst % SHARD) // 128
    blk = src // BLK
    key = (core * NBLK + blk) * NT + tloc
    # secondary sort by src within each group: ascending gather addresses
    # give much better HBM locality for the 256B random reads
    order = np.argsort(key * BLK + (src - blk * BLK), kind="stable")
    s_src = src[order]
    s_dst = dst[order]

    ngroups = C * NBLK * NT
    cnt = np.bincount(key, minlength=ngroups).reshape(C, NBLK, NT)
    starts = np.zeros(ngroups + 1, dtype=np.int64)
    np.cumsum(cnt.reshape(-1), out=starts[1:])

    lay = Layout(cfg, cnt.max(axis=0))
    GT = lay.GT

    per_core = []
    for c in range(C):
        idx_stream = np.zeros(lay.ntok, dtype=np.int16)
        dloc_stream = np.full(lay.ntok, 9999.0, dtype=np.float32)
        for b in range(NBLK):
            for t in range(NT):
                g = (c * NBLK + b) * NT + t
                a, e = starts[g], starts[g + 1]
                n = e - a
                pos = lay.off[b, t]
                idx_stream[pos:pos + n] = (s_src[a:e] - b * BLK).astype(np.int16)
                dloc_stream[pos:pos + n] = (
                    s_dst[a:e] - (c * SHARD + (t // GT) * GT * 128)
                ).astype(np.float32)
        idx_rep = np.ascontiguousarray(
            np.tile(idx_stream.reshape(-1, 16).T, (8, 1))
        )  # [128, ntok//16]
        dloc_w = np.ascontiguousarray(
            dloc_stream.reshape(-1, 128).T
        ).astype(np.float16)  # [128, nktot]
        per_core.append({"idx": idx_rep, "dloc": dloc_w})

    return deg, lay, per_core


def _wrap_deg(deg, cfg: CFG):
    """degB [128, NBLK*NBT] (block-wrapped, pad 1.0) and per-core degS
    [128, NT] (shard-wrapped, pad 1.0)."""
    N, NBLK, BLK, NBT = cfg.N, cfg.NBLK, cfg.BLK, cfg.NBT
    C, SHARD, NT = cfg.C, cfg.SHARD, cfg.NT
    degB = np.ones((128, NBLK * NBT), dtype=np.float32)
    for b in range(NBLK):
        for j in range(NBT):
            base = b * BLK + j * 128
            m = min(128, (b + 1) * BLK - base, N - base)
            if m > 0:
                degB[:m, b * NBT + j] = deg[base:base + m]
    degS = np.ones((C, 128, NT), dtype=np.float32)
    for c in range(C):
        for t in range(NT):
            base = c * SHARD + t * 128
            m = min(128, (c + 1) * SHARD - base)
            degS[c, :m, t] = deg[base:base + m]
    return degB, degS


# ------------------------------------------------------------ bass program --
def _build(cfg: CFG, lay: Layout, ablate=()):
    import concourse.bacc as bacc
    import concourse.mybir as mybir
    from concourse import tile

    f32 = mybir.dt.float32
    f16 = mybir.dt.float16
    i16 = mybir.dt.int16
    i32 = mybir.dt.int32
    ALU = mybir.AluOpType
    ACTF = mybir.ActivationFunctionType

    N, F, H, O, C = cfg.N, cfg.F, cfg.H, cfg.O, cfg.C
    NBLK, BLK, NBT = cfg.NBLK, cfg.BLK, cfg.NBT
    NT, SHARD, LASTV, HP = cfg.NT, cfg.SHARD, cfg.LASTV, cfg.HP
    CH_KT, SLAB, STG = cfg.CH_KT, cfg.SLAB, cfg.STG

    nktot = lay.nktot
    ntok = lay.ntok
    blk_kt_base = lay.blk_kt_base
    GT = lay.GT
    v3 = isinstance(lay, Layout3)

    nc = bacc.Bacc("TRN2", target_bir_lowering=False, debug=False,
                   num_devices=cfg.C,
                   dynamic_dma_scratch_size=cfg.DMA_SCRATCH,
                   num_swdge_queues=4)

    # ---- DRAM I/O
    shard_build_pre = v3 and "oldbuild" not in ablate
    xT_d = (None if shard_build_pre else
            nc.dram_tensor("xT", [F, N], f32, kind="ExternalInput"))
    xTs_d = (nc.dram_tensor("xTs", [F, SHARD], f32, kind="ExternalInput")
             if v3 else None)
    degB_d = (None if shard_build_pre else
              nc.dram_tensor("degB", [128, NBLK * NBT], f32,
                             kind="ExternalInput"))
    degS_d = nc.dram_tensor("degS", [128, NT], f32, kind="ExternalInput")
    idx_d = nc.dram_tensor("idx", [128, ntok // 16], i16, kind="ExternalInput")
    dloc_d = nc.dram_tensor("dloc", [128, nktot], f16, kind="ExternalInput")
    W1_d = nc.dram_tensor("W1", [F, H], f32, kind="ExternalInput")
    W2_d = nc.dram_tensor("W2", [H, O], f32, kind="ExternalInput")
    b1_d = nc.dram_tensor("b1", [1, H], f32, kind="ExternalInput")
    b2_d = nc.dram_tensor("b2", [1, O], f32, kind="ExternalInput")
    P_d = nc.dram_tensor("P", [1, O], f32, kind="ExternalInput")
    K_d = nc.dram_tensor("K", [1, O], f32, kind="ExternalInput")
    U_d = nc.dram_tensor("U", [1, O], f32, kind="ExternalInput")
    out_d = nc.dram_tensor("out", [SHARD, O], f32, kind="ExternalOutput")

    TW = 128  # f16 table row width (256B gather granule; cols >= H unused)
    TWv = 2 * TW if "elem512" in ablate else TW
    shard_build = v3 and "oldbuild" not in ablate
    if shard_build:
        t1loc = nc.dram_tensor("t1loc", [SHARD, TWv], f16)
        tab1f = nc.dram_tensor("tab1f", [N, TWv], f16, addr_space="Shared")
        tab1 = [tab1f[b * BLK: b * BLK + min(BLK, N - b * BLK), :]
                for b in range(NBLK)]
    else:
        tab1 = [
            nc.dram_tensor(f"tab1_{b}", [min(BLK, N - b * BLK), TWv], f16)
            for b in range(NBLK)
        ]
    t2loc = nc.dram_tensor("t2loc", [SHARD, TWv], f16)
    tab2 = nc.dram_tensor("tab2", [N, TWv], f16, addr_space="Shared")

    with tile.TileContext(nc, num_cores=C) as tc:
        with (
            tc.tile_pool(name="const", bufs=1) as const,
            tc.tile_pool(name="xslab", bufs=2) as xpool,
            tc.tile_pool(name="t1st", bufs=2) as t1pool,
            tc.tile_pool(name="g", bufs=16) as gpool,
            tc.tile_pool(name="s", bufs=8) as spool,
            tc.tile_pool(name="work", bufs=3) as work,
            tc.tile_pool(name="post", bufs=2) as post,
            tc.tile_pool(name="ost", bufs=2) as opool,
            tc.tile_pool(name="ps_seg", bufs=3, space="PSUM") as ps_seg,
            tc.tile_pool(name="ps_bld", bufs=1, space="PSUM") as ps_bld,
            tc.tile_pool(name="ps_tr", bufs=2, space="PSUM") as ps_tr,
            tc.tile_pool(name="ps_t2", bufs=2, space="PSUM") as ps_t2,
        ):
            # ---------------- constants / small inputs
            iota_i = const.tile([128, GT * 128], i32)
            nc.gpsimd.iota(iota_i[:, :], pattern=[[128, GT], [1, 128]],
                           base=0, channel_multiplier=0)
            IOTA16 = const.tile([128, GT * 128], f16)
            nc.vector.tensor_copy(IOTA16[:, :], iota_i[:, :])
            IOTA = IOTA16  # first 128 columns are a plain 0..127 iota row
            IDiota = const.tile([128, 128], f32)
            pidx_i = const.tile([128, 1], i32)
            nc.gpsimd.iota(pidx_i[:, :], pattern=[[0, 1]], base=0,
                           channel_multiplier=1)
            PIDX = const.tile([128, 1], f32)
            nc.vector.tensor_copy(PIDX[:, :], pidx_i[:, :])
            ID = const.tile([128, 128], f32)
            nc.vector.tensor_copy(IDiota[:, :], iota_i[:, :128])
            nc.vector.tensor_scalar(out=ID[:, :], in0=IDiota[:, :],
                                    scalar1=PIDX[:, :], scalar2=None,
                                    op0=ALU.is_equal)

            W1s = const.tile([F, H], f32)
            nc.sync.dma_start(W1s[:, :], W1_d[:, :])
            W2s = const.tile([H, O], f32)
            nc.sync.dma_start(W2s[:, :], W2_d[:, :])

            b1row = const.tile([1, H], f32)
            nc.sync.dma_start(b1row[:, :], b1_d[:, :])
            BIAS1 = const.tile([128, H], f32)
            nc.gpsimd.partition_broadcast(BIAS1[:, :], b1row[:, :])

            b2row = const.tile([1, O], f32)
            nc.sync.dma_start(b2row[:, :], b2_d[:, :])
            prow = const.tile([1, O], f32)
            nc.sync.dma_start(prow[:, :], P_d[:, :])
            krow = const.tile([1, O], f32)
            nc.sync.dma_start(krow[:, :], K_d[:, :])
            urow = const.tile([1, O], f32)
            nc.sync.dma_start(urow[:, :], U_d[:, :])
            pku = const.tile([1, O], f32)
            nc.vector.tensor_mul(pku[:, :], prow[:, :], krow[:, :])
            nc.vector.tensor_mul(pku[:, :], pku[:, :], urow[:, :])
            nc.vector.tensor_add(pku[:, :], pku[:, :], b2row[:, :])
            BIAS2 = const.tile([128, O], f32)
            nc.gpsimd.partition_broadcast(BIAS2[:, :], pku[:, :])

            if degB_d is not None:
                degB = const.tile([128, NBLK * NBT], f32)
                nc.sync.dma_start(degB[:, :], degB_d[:, :])
                dinvB = const.tile([128, NBLK * NBT], f32)
                nc.vector.reciprocal(dinvB[:, :], degB[:, :])
                nc.scalar.sqrt(dinvB[:, :], dinvB[:, :])

            degS = const.tile([128, NT], f32)
            nc.sync.dma_start(degS[:, :], degS_d[:, :])
            dinvS = const.tile([128, NT], f32)
            nc.vector.reciprocal(dinvS[:, :], degS[:, :])
            nc.scalar.sqrt(dinvS[:, :], dinvS[:, :])

            idxS = const.tile([128, ntok // 16], i16)
            nc.sync.dma_start(idxS[:, :], idx_d[:, :])
            dloc = const.tile([128, nktot], f16)
            nc.sync.dma_start(dloc[:, :], dloc_d[:, :])

            agg = const.tile([128, NT, H], f32)
            if v3:
                # Own-shard pass over dinv*(x@W1): initializes agg with the
                # self-loop table rows AND (shard_build) stages this core's
                # slice of the layer-1 table for the AllGather.
                for s0 in range(0, SHARD, SLAB):
                    w = min(SLAB, SHARD - s0)
                    xs = xpool.tile([F, SLAB], f32, tag="xss")
                    nc.sync.dma_start(xs[:, :w], xTs_d[:, s0:s0 + w])
                    st = None
                    if shard_build:
                        st = t1pool.tile([128, _cdiv(SLAB, 128), H], f16,
                                         tag="t1st")
                    nfull = 0
                    for j0 in range(0, w, 128):
                        m = min(128, w - j0)
                        t = (s0 + j0) // 128
                        ps = ps_bld.tile([128, H], f32, tag="psb")
                        if m < 128:
                            # pad rows: zero so downstream stays finite
                            nc.vector.memset(agg[:, t, :], 0.0)
                        nc.tensor.matmul(ps[:m, :], lhsT=xs[:, j0:j0 + m],
                                         rhs=W1s[:, :], start=True, stop=True)
                        nc.scalar.activation(
                            agg[:m, t, :], ps[:m, :], ACTF.Copy,
                            scale=dinvS[:m, t:t + 1])
                        if shard_build:
                            nc.scalar.activation(
                                st[:m, j0 // 128, :], ps[:m, :], ACTF.Copy,
                                scale=dinvS[:m, t:t + 1])
                            if m == 128:
                                nfull += 1
                    if shard_build:
                        if nfull:
                            dst_ap = t1loc[s0:s0 + nfull * 128, :H].rearrange(
                                "(j p) f -> p j f", p=128)
                            nc.sync.dma_start(dst_ap, st[:, :nfull, :])
                        if nfull * 128 < w:
                            mm = w - nfull * 128
                            nc.sync.dma_start(
                                t1loc[s0 + nfull * 128: s0 + w, :H],
                                st[:mm, nfull, :])
                if shard_build:
                    nc.gpsimd.collective_compute(
                        "AllGather", mybir.AluOpType.bypass,
                        replica_groups=[list(range(C))],
                        ins=[t1loc[:, :].opt()],
                        outs=[tab1f[:, :].opt()])
            else:
                nc.vector.memset(agg[:, :, :], 0.0)

            # ---------------- layer-1 message table: tab1_b = dinv*(x@W1)
            def build_table1(b):
                nodes_b = min(BLK, N - b * BLK)
                for s0 in range(0, nodes_b, SLAB):
                    w = min(SLAB, nodes_b - s0)
                    xs = xpool.tile([F, SLAB], f32, tag="xs")
                    nc.sync.dma_start(xs[:, :w],
                                      xT_d[:, b * BLK + s0: b * BLK + s0 + w])
                    st = t1pool.tile([128, _cdiv(SLAB, 128), H], f16, tag="t1st")
                    nfull = 0
                    for j0 in range(0, w, 128):
                        m = min(128, w - j0)
                        jt = (s0 + j0) // 128  # node-tile idx within block
                        ps = ps_bld.tile([128, H], f32, tag="psb")
                        nc.tensor.matmul(ps[:m, :], lhsT=xs[:, j0:j0 + m],
                                         rhs=W1s[:, :], start=True, stop=True)
                        nc.scalar.activation(
                            st[:m, j0 // 128, :], ps[:m, :], ACTF.Copy,
                            scale=dinvB[:m, b * NBT + jt: b * NBT + jt + 1])
                        if m == 128:
                            nfull += 1
                    # store staged tiles to DRAM
                    if nfull:
                        dst_ap = tab1[b][s0:s0 + nfull * 128, :H].rearrange(
                            "(j p) f -> p j f", p=128)
                        nc.sync.dma_start(dst_ap, st[:, :nfull, :])
                    if nfull * 128 < w:  # ragged tail tile of the block
                        m = w - nfull * 128
                        nc.sync.dma_start(
                            tab1[b][s0 + nfull * 128: s0 + w, :H],
                            st[:m, nfull, :])

            if "nobuild" not in ablate and not shard_build:
                for b in range(NBLK):
                    build_table1(b)

            # ---------------- gather + segment-sum matmul for one layer
            MAXKB = 8  # S-matrices built per DVE instruction
            qrot = [0]  # SWDGE queue rotation across gather calls

            def seg_layer(table_aps, uw):
                """table_aps[b]: block b's [rows, TW] f16 message rows; only
                the first uw columns are meaningful."""
                for b in range(NBLK):
                    kt_in_blk = blk_kt_base[b + 1] - blk_kt_base[b]
                    if kt_in_blk == 0:
                        continue
                    # gather chunks
                    gtiles = []
                    SP = "singlepacket" in ablate
                    for ci in range(_cdiv(kt_in_blk, CH_KT)):
                        kts = min(CH_KT, kt_in_blk - ci * CH_KT)
                        g = gpool.tile([128, CH_KT, TWv], f16, tag="g")
                        tok0 = (blk_kt_base[b] + ci * CH_KT) * 128
                        if "smallgather" in ablate:
                            nc.gpsimd.dma_gather(
                                g[:, :1, :], table_aps[b],
                                idxS[:, tok0 // 16: (tok0 + 128) // 16],
                                num_idxs=128, num_idxs_reg=128,
                                elem_size=TWv, single_packet=SP,
                                queue_num=qrot[0] % 4)
                        elif "smallreg" in ablate:
                            nc.gpsimd.dma_gather(
                                g[:, :kts, :], table_aps[b],
                                idxS[:, tok0 // 16: (tok0 + kts * 128) // 16],
                                num_idxs=kts * 128, num_idxs_reg=128,
                                elem_size=TWv, single_packet=SP,
                                queue_num=qrot[0] % 4)
                        else:
                            nc.gpsimd.dma_gather(
                                g[:, :kts, :], table_aps[b],
                                idxS[:, tok0 // 16: (tok0 + kts * 128) // 16],
                                num_idxs=kts * 128, num_idxs_reg=kts * 128,
                                elem_size=TWv, single_packet=SP,
                                queue_num=qrot[0] % 4)
                        qrot[0] += 1
                        gtiles.append(g)

                    def gslice(kglob):
                        ci, sl = divmod(kglob - blk_kt_base[b], CH_KT)
                        return gtiles[ci][:, sl, :uw]

                    # consume: per dst-tile, its token range [o0, o1) in the
                    # common layout; K-tiles at supertile boundaries are
                    # shared between adjacent dst-tiles (S masks the others).
                    if "noconsume" in ablate:
                        continue
                    for t in range(NT):
                        if v3:
                            r = lay.kr[(b, t)]
                            if r is None:
                                continue
                            k0, k1 = r
                        else:
                            ct = int(lay.ctok[b, t])
                            if ct == 0:
                                continue
                            o0 = int(lay.off[b, t])
                            o1 = o0 + ct
                            k0, k1 = o0 // 128, (o1 - 1) // 128
                        it = t % GT  # iota variant within supertile
                        ps = ps_seg.tile([128, uw], f32, tag="pss")
                        k = k0
                        while k <= k1:
                            kb = min(MAXKB, k1 + 1 - k)
                            Sb = spool.tile([128, MAXKB, 128], f16, tag="s")
                            nc.vector.tensor_tensor(
                                out=Sb[:, :kb, :],
                                in0=IOTA16[:, it * 128:(it + 1) * 128]
                                    .unsqueeze(1)
                                    .broadcast_to([128, kb, 128]),
                                in1=dloc[:, k:k + kb].unsqueeze(2)
                                    .broadcast_to([128, kb, 128]),
                                op=ALU.is_equal)
                            for j in range(kb):
                                nc.tensor.matmul(
                                    ps[:, :], lhsT=Sb[:, j, :],
                                    rhs=gslice(k + j),
                                    start=(k + j == k0),
                                    stop=(k + j == k1))
                            k += kb
                        nc.vector.tensor_add(agg[:, t, :uw],
                                             agg[:, t, :uw], ps[:, :])

            def seg_layer_tb(table_aps, uw):
                """order='Tb': per supertile, gather all 4 block groups,
                then one PSUM accumulation per dst-tile across blocks."""
                SP = "singlepacket" in ablate
                for T in range(lay.NSUP):
                    gmap = {}
                    for b in range(NBLK):
                        off = int(lay.off_group[b, T])
                        kt_grp = int(lay.glen[b, T]) // 128
                        k_base = off // 128
                        ci = 0
                        while ci < kt_grp:
                            kts = min(CH_KT, kt_grp - ci)
                            g = gpool.tile([128, CH_KT, TWv], f16, tag="g")
                            tok0 = (k_base + ci) * 128
                            nc.gpsimd.dma_gather(
                                g[:, :kts, :], table_aps[b],
                                idxS[:, tok0 // 16:
                                     (tok0 + kts * 128) // 16],
                                num_idxs=kts * 128, num_idxs_reg=kts * 128,
                                elem_size=TWv, single_packet=SP,
                                queue_num=qrot[0] % 4)
                            qrot[0] += 1
                            for s in range(kts):
                                gmap[k_base + ci + s] = (g, s)
                            ci += kts
                    if "noconsume" in ablate:
                        continue
                    t0, t1 = T * GT, min((T + 1) * GT, NT)
                    for t in range(t0, t1):
                        spans = [lay.kr[(b, t)] for b in range(NBLK)
                                 if lay.kr[(b, t)] is not None]
                        if not spans:
                            continue
                        it = t % GT
                        firstk = spans[0][0]
                        lastk = spans[-1][1]
                        ps = ps_seg.tile([128, uw], f32, tag="pss")
                        for (k0, k1) in spans:
                            k = k0
                            while k <= k1:
                                kb = min(MAXKB, k1 + 1 - k)
                                Sb = spool.tile([128, MAXKB, 128], f16,
                                                tag="s")
                                nc.vector.tensor_tensor(
                                    out=Sb[:, :kb, :],
                                    in0=IOTA16[:, it * 128:(it + 1) * 128]
                                        .unsqueeze(1)
                                        .broadcast_to([128, kb, 128]),
                                    in1=dloc[:, k:k + kb].unsqueeze(2)
                                        .broadcast_to([128, kb, 128]),
                                    op=ALU.is_equal)
                                for j in range(kb):
                                    gt_, sl = gmap[k + j]
                                    nc.tensor.matmul(
                                        ps[:, :], lhsT=Sb[:, j, :],
                                        rhs=gt_[:, sl, :uw],
                                        start=(k + j == firstk),
                                        stop=(k + j == lastk))
                                k += kb
                        nc.vector.tensor_add(agg[:, t, :uw],
                                             agg[:, t, :uw], ps[:, :])

            seg = seg_layer_tb if (v3 and lay.order == "Tb") else seg_layer

            # ---------------- layer 1
            tab1_aps = (tab1 if shard_build
                        else [tab1[b][:, :] for b in range(NBLK)])
            seg(tab1_aps, H)

            # post: h1 = relu(dinv*agg + b1); t2 = dinv*(h1@W2) padded
            def staged_store(dram, stile, grp, nt_in_grp, width):
                """store staging tile rows [grp*STG .. ) handling ragged tail"""
                t0 = grp * STG
                nfull = 0
                for tt in range(nt_in_grp):
                    if (t0 + tt) * 128 + 128 <= SHARD:
                        nfull += 1
                if nfull:
                    dst = dram[t0 * 128: t0 * 128 + nfull * 128,
                               :width].rearrange("(j p) f -> p j f", p=128)
                    nc.sync.dma_start(dst, stile[:, :nfull, :width])
                if nfull < nt_in_grp:
                    nc.sync.dma_start(
                        dram[(t0 + nfull) * 128: SHARD, :width],
                        stile[:LASTV, nfull, :width])

            for grp in range(_cdiv(NT, STG)):
                nt_in_grp = min(STG, NT - grp * STG)
                st = post.tile([128, STG, H], f16, tag="t2st")
                # cols O:H of t2loc/tab2 are never read (the L2 consume
                # matmul rhs spans only :O), so no zeroing is needed
                for tt in range(nt_in_grp):
                    t = grp * STG + tt
                    h1 = work.tile([128, H], f32, tag="h1")
                    nc.vector.scalar_tensor_tensor(
                        out=h1[:, :], in0=agg[:, t, :],
                        scalar=dinvS[:, t:t + 1], in1=BIAS1[:, :],
                        op0=ALU.mult, op1=ALU.add)
                    nc.scalar.activation(h1[:, :], h1[:, :], ACTF.Relu)
                    pst = ps_tr.tile([H, 128], f32, tag="pstr")
                    nc.tensor.transpose(pst[:, :], h1[:, :], ID[:, :])
                    h1t = work.tile([H, 128], f32, tag="h1t")
                    nc.scalar.copy(h1t[:, :], pst[:, :])
                    ps2 = ps_t2.tile([128, O], f32, tag="pst2")
                    nc.tensor.matmul(ps2[:, :], lhsT=h1t[:, :], rhs=W2s[:, :],
                                     start=True, stop=True)
                    nc.scalar.activation(st[:, tt, :O], ps2[:, :], ACTF.Copy,
                                         scale=dinvS[:, t:t + 1])
                    if v3:
                        # L2 self-loop init: agg[:, t, :O] = dinv*(h1@W2)
                        # (own t2 table row; postproc applies dst-side dinv)
                        nc.scalar.activation(
                            agg[:, t, :O], ps2[:, :], ACTF.Copy,
                            scale=dinvS[:, t:t + 1])
                staged_store(t2loc, st, grp, nt_in_grp, H)

            # ---------------- exchange layer-2 table
            nc.gpsimd.collective_compute(
                "AllGather", mybir.AluOpType.bypass,
                replica_groups=[list(range(C))],
                ins=[t2loc[:, :].opt()],
                outs=[tab2[:, :].opt()])
            if "agx2" in ablate:  # probe: cost of one extra AllGather
                nc.gpsimd.collective_compute(
                    "AllGather", mybir.AluOpType.bypass,
                    replica_groups=[list(range(C))],
                    ins=[t2loc[:, :].opt()],
                    outs=[tab2[:, :].opt()])

            # ---------------- layer 2
            if not v3:
                nc.vector.memset(agg[:, :, :], 0.0)
            tab2_aps = [tab2[b * BLK: b * BLK + min(BLK, N - b * BLK), :]
                        for b in range(NBLK)]
            seg(tab2_aps, O)

            # post: y = relu(dinv*agg + b2 + pku); out = log_softmax(y)
            for grp in range(0 if "nopost2" not in ablate
                             else _cdiv(NT, STG), _cdiv(NT, STG)):
                nt_in_grp = min(STG, NT - grp * STG)
                st = opool.tile([128, STG, O], f32, tag="ost")
                for tt in range(nt_in_grp):
                    t = grp * STG + tt
                    y = work.tile([128, O], f32, tag="y")
                    nc.vector.scalar_tensor_tensor(
                        out=y[:, :], in0=agg[:, t, :O],
                        scalar=dinvS[:, t:t + 1], in1=BIAS2[:, :],
                        op0=ALU.mult, op1=ALU.add)
                    nc.scalar.activation(y[:, :], y[:, :], ACTF.Relu)
                    nmax = work.tile([128, 1], f32, tag="nmax")
                    nc.vector.tensor_reduce(nmax[:, :], y[:, :],
                                            axis=mybir.AxisListType.X,
                                            op=ALU.max, negate=True)
                    ex = work.tile([128, O], f32, tag="ex")
                    esum = work.tile([128, 1], f32, tag="esum")
                    nc.scalar.activation(ex[:, :], y[:, :], ACTF.Exp,
                                         bias=nmax[:, :], scale=1.0,
                                         accum_out=esum[:, :])
                    lsum = work.tile([128, 1], f32, tag="lsum")
                    nc.scalar.activation(lsum[:, :], esum[:, :], ACTF.Ln)
                    nc.vector.tensor_scalar(
                        out=st[:, tt, :], in0=y[:, :], scalar1=nmax[:, :],
                        scalar2=lsum[:, :], op0=ALU.add, op1=ALU.subtract)
                staged_store(out_d, st, grp, nt_in_grp, O)

    nc.compile()
    return nc


# ------------------------------------------------------------------ entry --
def make_in_maps(inputs, cfg, per_core, degB, degS, xT, v3):
    in_maps = []
    for c in range(cfg.C):
        m = {
            "xT": xT,
            "degB": degB,
            "degS": np.ascontiguousarray(degS[c]),
            "idx": per_core[c]["idx"],
            "dloc": per_core[c]["dloc"],
            "W1": np.asarray(inputs["W1"], np.float32),
            "W2": np.asarray(inputs["W2"], np.float32),
            "b1": np.asarray(inputs["b1"], np.float32).reshape(1, -1),
            "b2": np.asarray(inputs["b2"], np.float32).reshape(1, -1),
            "P": np.asarray(inputs["P"], np.float32).reshape(1, -1),
            "K": np.asarray(inputs["K"], np.float32).reshape(1, -1),
            "U": np.asarray(inputs["U"], np.float32).reshape(1, -1),
        }
        if v3:
            m["xTs"] = np.ascontiguousarray(
                xT[:, c * cfg.SHARD:(c + 1) * cfg.SHARD])
        in_maps.append(m)
    return in_maps


def prepare_and_run(inputs, cfg=None, trace=False, v3=True, **run_kwargs):
    """Preprocess, build, run on 8 cores.  Returns (out, BassKernelResults)."""
    from concourse.bass_utils import run_bass_kernel_spmd

    cfg = cfg or CFG()
    x = np.asarray(inputs["x"], dtype=np.float32)
    edge_index = np.asarray(inputs["edge_index"])

    pre = _preprocess_v3 if v3 else _preprocess
    deg, lay, per_core = pre(edge_index, cfg)
    degB, degS = _wrap_deg(deg, cfg)
    xT = np.ascontiguousarray(x.T)

    nc = _build(cfg, lay)

    in_maps = make_in_maps(inputs, cfg, per_core, degB, degS, xT, v3)
    res = run_bass_kernel_spmd(nc, in_maps, core_ids=list(range(cfg.C)),
                               trace=trace, **run_kwargs)
    out = np.concatenate([res.results[c]["out"] for c in range(cfg.C)], axis=0)
    return out.astype(np.float32), res


def kernel(**inputs):
    out, _ = prepare_and_run(inputs)
    return out


if __name__ == "__main__":
    import reference

    inputs = {k: np.asarray(v) for k, v in reference.setup_inputs().items()}
    got = kernel(**inputs)
    want = np.asarray(reference.reference(**inputs))
    err = np.abs(got - want).max() / max(np.abs(want).max(), 1e-9)
    print("rel err:", err)

